# revision 79
# baseline (speedup 1.0000x reference)
"""Trainium2 Bass kernel for the dynamic-attention-block CNN (nn_DAB).

Data-parallel over batch: 8 samples -> 8 NeuronCores. Each core runs the
full per-sample network with activations resident in SBUF as padded
"frames": 128 partitions = 64 channels x 2 image halves, each half a
98x196 zero-padded row-major frame (rows -1..96 / 95..192 of the 192x192
image, cols -2..193).

Conv structure (all single 128-partition matmuls; the two image halves
ride in one instruction via block-diagonal weights):
  - 3x3 convs and dynamic depthwise convs run in fp8e4m3 DoubleRow mode:
    taps are processed in pairs (lhsT [128,2,128], rhs [128,2,N] with the
    pair dim striding between the two tap offsets), 5 passes per conv.
  - the 1x1 convs are also fp8 DoubleRow via stride-0 pairs (second pair
    slot zero-weighted), halving their cost; their rhs t1 = prelu(dw
    psum) is stored in fp8 with the dw kernel pre-scale KS=16 riding
    along (prelu is positively homogeneous; KS=16 keeps t1 in fp8 range)
    and divided out of the 1x1 weights.
  - channel-attention gates are bf16 diagonal matmuls accumulating
    x*att into the same PSUM group as the 1x1 (no vector gating pass);
    the conv3 residual +x is instead an in-place GPSIMD tensor_tensor
    add on the drained f32 output tiles.
  - the additive 32x32-upsampled map is folded into conv2 as one fp8
    DoubleRow pass: 18 partitions hold the 9 tap-shifted copies of the
    upsampled map per half, weights are the channel-summed conv2 taps.

Schedule: two software-pipelined wavefront blocks with per-stage tile
rotation (stage k starts at tile 2k, so each stage's wrap tiles land
long after the previous stage produced their halo rows, which are
DMA'd as soon as source tiles 47/0 drain):
  block A = da1 + conv1 (lag 8) + conv2 (lag 16)
  block B = da2 + conv3 (lag 8)
Interleaving adjacent stages keeps every drain engine below the PE
rate (a lone da stage is drain-bound) and lets conv3's f32 output
stream (~26us of serialized DMA-engine time) overlap da2's compute.
PSUM drains alternate between Act (1-op activation with scale+bias)
and DVE (two-scalar tensor_scalar + SBUF-only prelu via
scalar_tensor_tensor, which cannot read PSUM); the fp8 frame copy for
da2 alternates DVE/GPSIMD.  Output rides in 2-tile (4-row) groups (the
final 8 tiles in two dedicated 4-tile groups, shortening the
end-of-stream chain), two DMAs per group on the otherwise-idle sync
queue.  Input DMAs ride one
queue in consumption order (small kd1/1x1 weight pack first, then
interleaved x8/xb row chunks) so da1 never outruns the stream.
"""

import sys

for _p in ("/opt/trn_rl_repo", "/root/.axon_site/_ro/pypackages"):
    if _p not in sys.path:
        sys.path.insert(0, _p)

import numpy as np
import ml_dtypes

BF16 = ml_dtypes.bfloat16
F8 = ml_dtypes.float8_e4m3

B, C, H, W = 8, 64, 192, 192
HW = H * W
FR, FC = 98, 196          # frame rows / cols per half
FF = FR * FC              # frame elems per partition
ALPHA = 0.1               # leaky slope
WS = 16.0                 # fp8 conv weight pre-scale
KS = 16.0                 # fp8 dw kernel pre-scale (also bounds fp8 t1)
SC = 256.0                # da-stage PSUM scale: psB = SC*(1x1 out + att*x)

# bias pack columns (the *W columns are pre-scaled by WS / SC for DVE
# drains, which add the bias before the descale multiply; Act drains
# scale first)
(BI_B1, BI_B1W, BI_B2, BI_B2W, BI_B3, BI_B3W,
 BI_CB1, BI_CB1S, BI_CB2, BI_CB2S) = range(10)

# DoubleRow tap pairing: (tap_a, tap_b) with taps t = 3*dy + dx,
# delta(t) = (dy-1)*FC + (dx-1).  5 passes cover all 9 taps; the last
# pass's second slot has zero weight (stride 0 keeps the read in-bounds).
PAIRS = [(0, 1), (3, 4), (6, 7), (2, 5), (8, None)]

# packed fp8 weight slots: wkd carries what da1's first tiles need
# (kd1 + the two 1x1 stride-0 DoubleRow pairs) as a small early DMA;
# wp8 carries the rest
WK_KD1, WK_CW1, WK_CW2 = 0, 5, 6          # rows in wkd [128, 7, 2, 128]
WP_W1, WP_W2, WP_KD2, WP_W3 = range(4)    # 5-row slots in wp8
# packed bf16 weight slots in wpackb
WB_G1, WB_G2, WB_RID = range(3)

NTILE = 48
ROT = 2                   # per-stage tile-order rotation

_CACHE = {}


def _delta(t):
    return (t // 3 - 1) * FC + (t % 3 - 1)


def _build_nc():
    import concourse.bacc as bacc
    import concourse.mybir as mybir
    from concourse import tile

    f32 = mybir.dt.float32
    bf16 = mybir.dt.bfloat16
    f8 = mybir.dt.float8e4
    AF = mybir.ActivationFunctionType
    ALU = mybir.AluOpType
    DR = mybir.MatmulPerfMode.DoubleRow

    nc = bacc.Bacc("TRN2", target_bir_lowering=False, debug=False, num_devices=8)

    xb_d = nc.dram_tensor("xb", [128, FF], bf16, kind="ExternalInput").ap()
    x8_d = nc.dram_tensor("x8", [128, FF], f8, kind="ExternalInput").ap()
    wkd_d = nc.dram_tensor("wkd", [128, 7, 2, 128], f8, kind="ExternalInput").ap()
    wp8_d = nc.dram_tensor("wp8", [128, 20, 2, 128], f8, kind="ExternalInput").ap()
    wpb_d = nc.dram_tensor("wpb", [128, 3, 128], bf16, kind="ExternalInput").ap()
    wa_d = nc.dram_tensor("wa", [18, 2, 128], f8, kind="ExternalInput").ap()
    af_d = nc.dram_tensor("af", [18, FF], f8, kind="ExternalInput").ap()
    bias_d = nc.dram_tensor("bias", [128, 10], f32, kind="ExternalInput").ap()
    y_d = nc.dram_tensor("y", [C, HW], f32, kind="ExternalOutput").ap()

    # row-pair sweep tiles: 48 tiles of 2 image rows (392 frame cols)
    qt = [((2 * i + 1) * FC + 2, 2 * FC) for i in range(NTILE)]

    from contextlib import ExitStack
    with tile.TileContext(nc) as tc, ExitStack() as ctx:
        wpool = ctx.enter_context(tc.tile_pool(name="w", bufs=1))
        fbpool = ctx.enter_context(tc.tile_pool(name="fb", bufs=2))
        f8pool = ctx.enter_context(tc.tile_pool(name="f8", bufs=4))
        t1p = ctx.enter_context(tc.tile_pool(name="t1", bufs=4))
        tmpp = ctx.enter_context(tc.tile_pool(name="tmp", bufs=4))
        outp = ctx.enter_context(tc.tile_pool(name="outp", bufs=4))
        outb = ctx.enter_context(tc.tile_pool(name="outb", bufs=2))
        psA = ctx.enter_context(tc.tile_pool(name="psA", bufs=4, space="PSUM"))
        psB = ctx.enter_context(tc.tile_pool(name="psB", bufs=4, space="PSUM"))

        # ---- weights / constants to SBUF (batched DMAs) ----
        wkd = wpool.tile([128, 7, 2, 128], f8, tag="wkd")
        wp8 = wpool.tile([128, 20, 2, 128], f8, tag="wp8")
        wpb = wpool.tile([128, 3, 128], bf16, tag="wpb")
        wa = wpool.tile([18, 2, 128], f8, tag="wa")
        af = wpool.tile([18, FF], f8, tag="af")
        bias = wpool.tile([128, 10], f32, tag="bias")

        def w8(slot):
            return wp8[:, 5 * slot:5 * slot + 5, :, :]

        def wk5(row):
            return wkd[:, row:row + 5, :, :]

        def wcw(row):
            return wkd[:, row, :, :]

        def wb(slot):
            return wpb[:, slot, :]

        def cold(col):
            return bias[:, col:col + 1]

        # ---- input frames (host pre-padded); all bulk input DMAs ride
        # one queue in consumption order: weights first, then x8/xb
        # chunks interleaved by row coverage so da1 never outruns the
        # stream (transfers serialize on the DMA engines, so order is
        # what matters; the af map is only needed from conv2 onward) ----
        Xb = fbpool.tile([128, FF], bf16, tag="fb")
        X8 = f8pool.tile([128, FF], f8, tag="f8")
        O1 = f8pool.tile([128, FF], f8, tag="f8")
        O2 = f8pool.tile([128, FF], f8, tag="f8")
        O3b = fbpool.tile([128, FF], bf16, tag="fb")
        O38 = f8pool.tile([128, FF], f8, tag="f8")
        nc.sync.dma_start(out=wkd[...], in_=wkd_d)
        nc.gpsimd.dma_start(out=wpb[...], in_=wpb_d)
        nc.gpsimd.dma_start(out=wa[...], in_=wa_d)
        nc.gpsimd.dma_start(out=bias[...], in_=bias_d)
        # first fp8 rows split out so da1's tile 0 can start sooner
        nc.sync.dma_start(out=X8[:, 0:1176], in_=x8_d[:, 0:1176])
        nchunk = 8
        step = (FF + nchunk - 1) // nchunk
        for k in range(nchunk):
            c0, c1 = k * step, min((k + 1) * step, FF)
            if k == 0:
                nc.sync.dma_start(out=X8[:, 1176:c1], in_=x8_d[:, 1176:c1])
            else:
                nc.sync.dma_start(out=X8[:, c0:c1], in_=x8_d[:, c0:c1])
            nc.sync.dma_start(out=Xb[:, c0:c1], in_=xb_d[:, c0:c1])
            if k == 1:
                nc.sync.dma_start(out=wp8[...], in_=wp8_d)
        nc.scalar.dma_start(out=af[...], in_=af_d)

        # ---- PE warmup: throwaway matmuls while the input DMAs stream
        # in; keeps the p-state ramp finished before real work ----
        wrm = wpool.tile([128, 128], bf16, tag="wrm")
        nc.vector.memset(wrm[:, :], 0.0)
        pw = psA.tile([128, 128], f32, tag="psA")
        for _ in range(40):
            nc.tensor.matmul(pw[:, :], wrm[:, :], wrm[:, :],
                             start=True, stop=True, skip_group_check=True)

        def v3(m):
            return m[:, :].rearrange("p (a b) -> p a b", b=FC)

        # one-time pad zeroing for frame buffers not filled by host DMA.
        # Interior writes never touch pads again, so pads stay zero across
        # all later reuses of these pool buffers.
        for m in (O1, O2, O3b, O38):
            mv = v3(m)
            nc.gpsimd.memset(mv[0:64, 0, :], 0.0)
            nc.gpsimd.memset(mv[64:128, FR - 1, :], 0.0)
            nc.gpsimd.memset(mv[:, :, 0:2], 0.0)
            nc.gpsimd.memset(mv[:, :, FC - 2:FC], 0.0)

        def halo_a(m):
            # half1 top halo row (img 95) <- half0 frame row 96, src tile 47
            mv = v3(m)
            nc.sync.dma_start(out=mv[64:128, 0, :], in_=mv[0:64, 96, :])

        def halo_b(m):
            # half0 bottom halo row (img 96) <- half1 frame row 1, src tile 0
            mv = v3(m)
            nc.sync.dma_start(out=mv[0:64, FR - 1, :], in_=mv[64:128, 1, :])

        def maybe_halo(t, frames):
            if t == 47:
                for m in frames:
                    halo_a(m)
            elif t == 0:
                for m in frames:
                    halo_b(m)

        def order(stage):
            s = (ROT * stage) % NTILE
            return [(s + i) % NTILE for i in range(NTILE)]

        def dr_rhs(m8, q, n, pair):
            ta, tb = pair
            base = q + _delta(ta)
            stride = 0 if tb is None else _delta(tb) - _delta(ta)
            n = min(n, FF - base - max(stride, 0))
            r = m8[:, base:base + 1].copy()
            r.ap[1] = [stride, 2]
            r.ap.append([1, n])
            return r, n

        def conv_dr(ps, wsb, m8, q, n, stop=True):
            # P0 (top-left taps) never clamps, so it is the start pass and
            # always covers the full tile; clamped later passes only lose
            # tail columns that are pad positions, never emitted.
            for p in range(5):
                rhs, np_ = dr_rhs(m8, q, n, PAIRS[p])
                nc.tensor.matmul(ps[:, :np_], wsb[:, p, :, :], rhs,
                                 start=(p == 0), stop=(stop and p == 4),
                                 perf_mode=DR, skip_group_check=True)

        def iview(dst, q):
            # interior-only view: rows of the pair, cols 2:194
            r = q // FC
            return v3(dst)[:, r:r + 2, 2:194]

        def pview(src_ps, n):
            return src_ps[:, :n].rearrange("p (a b) -> p a b", b=FC)[:, :, 0:192]

        # Per-stage drain-engine alternation: each stage's per-engine drain
        # rate must stay below the PE rate or the drain queue backlog
        # throttles PSUM-buffer reuse (and the next stage's drains behind
        # it in the same queue).  Act drains are 1 op (~511ns); DVE prelu
        # drains are 2 ops (~800ns) since scalar_tensor_tensor can't read
        # PSUM, but plain identity+bias drains are 1 DVE op (~533ns).

        def prelu_drain_split(dst, ps, q, n, bw_col, scale):
            # ((psum + S*b) * 1/S) on DVE (bf16 staging), then an
            # SBUF-only prelu into the frame on GPSIMD, which is idle
            # during the da stages (scalar_tensor_tensor cannot read
            # PSUM, and a 2-op drain on DVE alone outruns the PE rate)
            tm = tmpp.tile([128, 2 * FC], bf16, tag="tm")
            tv = tm[:, :n].rearrange("p (a b) -> p a b", b=FC)[:, :, 0:192]
            nc.vector.tensor_scalar(tv, pview(ps, n), cold(bw_col),
                                    1.0 / scale, op0=ALU.add, op1=ALU.mult)
            nc.vector.scalar_tensor_tensor(iview(dst, q), tv, ALPHA, tv,
                                           op0=ALU.mult, op1=ALU.max)

        def make_da_stage(stage, inb, in8, kd_ap, cw_row, g_slot, cb_col,
                          cbs_col, out8):
            # Returns a step(i) driver, i in [0, NTILE+2): software-
            # pipelined by two tiles so the in-order PE queue rides out
            # the psA->Act t1->cw dependency chain (~1us).
            kd, cw, g = kd_ap, wcw(cw_row), wb(g_slot)
            tiles = order(stage)
            pipe = []

            def tail(prev, i):
                t1, q, n, t = prev
                pb = psB.tile([128, 2 * FC], f32, tag="psB")
                r = t1[:, 0:1].copy()
                r.ap[1] = [0, 2]
                r.ap.append([1, n])
                nc.tensor.matmul(pb[:, :n], cw, r, start=True, stop=False,
                                 perf_mode=DR, skip_group_check=True)
                nc.tensor.matmul(pb[:, :n], g, inb[:, q:q + n],
                                 start=False, stop=True, skip_group_check=True)
                if i % 5 in (0, 2):
                    nc.scalar.activation(iview(out8, q), pview(pb, n),
                                         AF.Prelu, scale=1.0 / SC,
                                         bias=cold(cb_col), alpha=ALPHA)
                else:
                    prelu_drain_split(out8, pb, q, n, cbs_col, SC)
                maybe_halo(t, (out8,))

            def step(i):
                if i < NTILE:
                    t = tiles[i]
                    q, n = qt[t]
                    pa = psA.tile([128, 2 * FC], f32, tag="psA")
                    conv_dr(pa, kd, in8, q, n)
                    if len(pipe) == 2:
                        tail(pipe.pop(0), i)
                    # t1 = prelu(psA) in fp8; the KS dw-weight scale rides
                    # along (prelu is positively homogeneous, KS=16 keeps
                    # the scaled values inside fp8 range) and is divided
                    # out of the fp8 1x1 weights on the host.
                    t1 = t1p.tile([128, 2 * FC], f8, tag="t1")
                    nc.scalar.activation(t1[:, :n], pa[:, :n], AF.Prelu,
                                         alpha=ALPHA)
                    pipe.append((t1, q, n, t))
                elif pipe:
                    tail(pipe.pop(0), i)

            return step

        # ---- network ----
        # da1 INTERLEAVED with conv1 (lag 8) and conv2 (lag 16): merging
        # the three stages' drain loads keeps Act and DVE below the
        # combined PE rate (da1 alone is drain-bound), and removes two
        # stage seams.  conv2 writes O38 while da1 still reads X8, so
        # O38 needs its own frame buffer (f8pool bufs=4).
        da1 = make_da_stage(0, Xb, X8, wk5(WK_KD1), WK_CW1, WB_G1, BI_CB1,
                            BI_CB1S, O1)
        LAG1, LAG2 = 8, 16
        c1tiles = order(1)
        c2tiles = order(2)
        for i in range(NTILE + LAG2):
            if i < NTILE + 2:
                da1(i)
            j = i - LAG1
            if 0 <= j < NTILE:
                t = c1tiles[j]
                q, n = qt[t]
                pa = psA.tile([128, 2 * FC], f32, tag="psA")
                conv_dr(pa, w8(WP_W1), O1, q, n)
                if j % 5 not in (1, 3):
                    nc.scalar.activation(iview(O2, q), pview(pa, n),
                                         AF.Prelu, scale=1.0 / WS,
                                         bias=cold(BI_B1), alpha=ALPHA)
                else:
                    prelu_drain_split(O2, pa, q, n, BI_B1W, WS)
                maybe_halo(t, (O2,))
            j2 = i - LAG2
            if j2 < 0:
                continue
            t = c2tiles[j2]
            q, n = qt[t]
            pa = psA.tile([128, 2 * FC], f32, tag="psA")
            conv_dr(pa, w8(WP_W2), O2, q, n, stop=False)
            r = af[:, q:q + 1].copy()
            r.ap[1] = [0, 2]
            r.ap.append([1, n])
            nc.tensor.matmul(pa[:, :n], wa[:, :, :], r,
                             start=False, stop=True, perf_mode=DR,
                             skip_group_check=True)
            if j2 % 2 == 0:
                nc.scalar.activation(iview(O3b, q), pview(pa, n), AF.Identity,
                                     scale=1.0 / WS, bias=cold(BI_B2))
                nc.gpsimd.tensor_copy(O38[:, q:q + n], O3b[:, q:q + n])
            else:
                nc.vector.tensor_scalar(iview(O3b, q), pview(pa, n),
                                        cold(BI_B2W), 1.0 / WS,
                                        op0=ALU.add, op1=ALU.mult)
                nc.vector.tensor_copy(O38[:, q:q + n], O3b[:, q:q + n])
            maybe_halo(t, (O38,))

        # ---- da2 INTERLEAVED with conv3 + residual (lag 8 positions):
        # conv3's outputs start streaming to DRAM ~30us earlier, so the
        # ~26us serialized output-DMA stream overlaps da2's compute
        # instead of trailing it, and the two stages' Act/DVE drain loads
        # merge to ~70-75% utilization each.  conv3: x (bf16, scaled by
        # WS via diag weights) and WS*b3 accumulate straight into PSUM;
        # drain with 1/WS.  Output rides in 4-tile (8-row) groups so each
        # group is just two DMAs, both on the otherwise-idle sync queue.
        # order(4) starts at tile 8 (4-aligned), so groups of 4
        # successive positions cover 4 consecutive tiles across the wrap.
        O4 = f8pool.tile([128, FF], f8, tag="f8")
        da2 = make_da_stage(3, O3b, O38, w8(WP_KD2), WK_CW2, WB_G2, BI_CB2,
                            BI_CB2S, O4)
        LAG3 = 8
        c3tiles = order(4)
        ot = None
        for i in range(NTILE + LAG3):
            if i < NTILE + 2:
                da2(i)
            j = i - LAG3
            if j < 0:
                continue
            t = c3tiles[j]
            q, n = qt[t]
            pa = psA.tile([128, 2 * FC], f32, tag="psA")
            conv_dr(pa, w8(WP_W3), O4, q, n)
            gsz = 2 if j < 40 else 4  # bigger final groups: fewer DMAs
            k = j % gsz                   # in the end-of-stream chain
            if k == 0:
                if gsz == 2:
                    ot = outp.tile([128, 4, 192], f32, tag="ot")
                else:
                    ot = outb.tile([128, 8, 192], f32, tag="otb")
                g0 = q // FC - 1  # image row of the group's first pair
            otv = ot[:, 2 * k:2 * k + 2, :]
            if j % 2 == 0 or j >= 40:
                nc.scalar.activation(otv, pview(pa, n), AF.Identity,
                                     scale=1.0 / WS, bias=cold(BI_B3))
            else:
                nc.vector.tensor_scalar(otv, pview(pa, n),
                                        cold(BI_B3W), 1.0 / WS,
                                        op0=ALU.add, op1=ALU.mult)
            # residual add in-place (mixed f32 += bf16): frees the PE from
            # the diag-matmul residual pass.  GPSIMD carries it during the
            # da2 overlap; in the final solo stretch the drain->resid
            # latency chain gates the last output DMAs, so keep it on the
            # faster DVE there (Act takes all the drains).
            if j >= 40:
                nc.vector.tensor_tensor(otv, otv, iview(Xb, q), op=ALU.add)
            else:
                nc.gpsimd.tensor_tensor(otv, otv, iview(Xb, q), op=ALU.add)
            if k % 2 == 1:
                # ship every 2 tiles (4 rows) even inside the big final
                # groups: the dedicated buffers avoid reuse contention
                # while fine DMA granularity keeps accumulation latency
                # out of the end-of-stream chain
                r0, r1 = 2 * (k - 1), 2 * (k + 1)
                h0, h1 = g0 + r0, g0 + r1
                nc.sync.dma_start(
                    out=y_d[:, h0 * 192:h1 * 192]
                    .rearrange("p (r c) -> p r c", c=192),
                    in_=ot[0:64, r0:r1, :])
                nc.sync.dma_start(
                    out=y_d[:, (96 + h0) * 192:(96 + h1) * 192]
                    .rearrange("p (r c) -> p r c", c=192),
                    in_=ot[64:128, r0:r1, :])

    nc.compile()
    return nc


def _pad_frame(xb, dtype):
    """(64,192,192) fp32 -> (128, FR*FC) dual-half padded frame."""
    fr = np.zeros((128, FR, FC), np.float32)
    fr[0:64, 1:97, 2:194] = xb[:, 0:96, :]
    fr[0:64, 97, 2:194] = xb[:, 96, :]
    fr[64:128, 1:97, 2:194] = xb[:, 96:192, :]
    fr[64:128, 0, 2:194] = xb[:, 95, :]
    return np.ascontiguousarray(fr.reshape(128, FF)).astype(dtype)


def _leaky_np(v):
    return np.where(v >= 0, v, ALPHA * v)


def _host_precompute(x, d, p):
    """Build per-core input maps. p: dict of raw weight arrays."""
    d = d.astype(np.float64)
    kern = {}
    att = {}
    for i in (1, 2):
        kw1, kw2 = p[f'da{i}_kw1'].astype(np.float64), p[f'da{i}_kw2'].astype(np.float64)
        ca1, ca2 = p[f'da{i}_ca1'].astype(np.float64), p[f'da{i}_ca2'].astype(np.float64)
        kern[i] = _leaky_np(d @ kw1.T) @ kw2.T          # (B, 576) [c*9+t]
        z = _leaky_np(d @ ca1.T) @ ca2.T
        att[i] = 1.0 / (1.0 + np.exp(-z))               # (B, 64)
    a32 = _leaky_np(d @ p['add_w1'].astype(np.float64).T) @ \
        p['add_w2'].astype(np.float64).T                # (B, 1024)

    cidx = np.arange(128) % 64

    def convw_dr(w):
        # (O, C, 3, 3) fp32 -> [128, 5, 2, 128] f8 block-diag DoubleRow taps
        wq = (w.astype(np.float32) * WS).astype(F8).astype(np.float32)
        wt = wq.transpose(1, 2, 3, 0).reshape(64, 9, 64)  # [c, t, o]
        out = np.zeros((128, 5, 2, 128), np.float32)
        for pi, (ta, tb) in enumerate(PAIRS):
            blk = np.zeros((64, 2, 64), np.float32)
            blk[:, 0, :] = wt[:, ta, :]
            if tb is not None:
                blk[:, 1, :] = wt[:, tb, :]
            out[0:64, pi, :, 0:64] = blk
            out[64:128, pi, :, 64:128] = blk
        return out.astype(F8)

    def cw_dr(w):
        # (O, C) -> [128, 2, 128] fp8 stride-0 DoubleRow pair: slot 0 is
        # the block-diag 1x1 weight scaled by SC/KS (t1 carries KS, the
        # drain divides SC back out), slot 1 is zero.
        out = np.zeros((128, 2, 128), np.float32)
        out[0:64, 0, 0:64] = w.T * (SC / KS)
        out[64:128, 0, 64:128] = w.T * (SC / KS)
        return out.astype(F8)

    # fp8 packed conv/dw weights (per-sample kd slots filled below)
    w1 = convw_dr(p['conv1_w'])
    w2 = convw_dr(p['conv2_w'])
    w3 = convw_dr(p['conv3_w'])
    cw1 = cw_dr(p['da1_cw'])
    cw2 = cw_dr(p['da2_cw'])

    # additive-map conv weights: wa[(h,t), 0, o_col] = WS * sum_c w2[o,c,t]
    w2sum = p['conv2_w'].astype(np.float64).sum(axis=1).reshape(64, 9)  # [o, t]
    wa = np.zeros((18, 2, 128), np.float32)
    for h in range(2):
        for t in range(9):
            wa[h * 9 + t, 0, h * 64:(h + 1) * 64] = WS * w2sum[:, t]
    wa = wa.astype(F8)

    rid = _diag128(np.full(128, WS, np.float32))

    maps = []
    for b in range(B):
        kd = {}
        for i in (1, 2):
            kc = (kern[i][b].reshape(64, 9).astype(np.float32) * KS) \
                .astype(F8).astype(np.float32)           # [c, t]
            kdl = np.zeros((128, 5, 2, 128), np.float32)
            for pi, (ta, tb) in enumerate(PAIRS):
                kdl[np.arange(128), pi, 0, np.arange(128)] = kc[cidx, ta]
                if tb is not None:
                    kdl[np.arange(128), pi, 1, np.arange(128)] = kc[cidx, tb]
            kd[i] = kdl.astype(F8)
        g = {i: _diag128(SC * att[i][b][cidx]) for i in (1, 2)}
        wkd = np.concatenate(
            [kd[1], cw1.reshape(128, 1, 2, 128),
             cw2.reshape(128, 1, 2, 128)], axis=1)
        wp8 = np.concatenate([w1, w2, kd[2], w3], axis=1)
        wpb = np.stack([g[1], g[2], rid], axis=1).astype(BF16)
        bias = np.zeros((128, 10), np.float32)
        bias[:, BI_B1] = p['conv1_b'][cidx]
        bias[:, BI_B1W] = WS * p['conv1_b'][cidx]
        bias[:, BI_B2] = p['conv2_b'][cidx]
        bias[:, BI_B2W] = WS * p['conv2_b'][cidx]
        bias[:, BI_B3] = p['conv3_b'][cidx]
        bias[:, BI_B3W] = WS * p['conv3_b'][cidx]
        bias[:, BI_CB1] = p['da1_cb'][cidx]
        bias[:, BI_CB1S] = SC * p['da1_cb'][cidx]
        bias[:, BI_CB2] = p['da2_cb'][cidx]
        bias[:, BI_CB2S] = SC * p['da2_cb'][cidx]

        # additive map frames: 18 partitions = 2 halves x 9 tap shifts
        a = a32[b].astype(np.float32).reshape(32, 32)
        aup = a[np.arange(192) // 6][:, np.arange(192) // 6]  # (192,192)
        afr = np.zeros((2, FF), np.float32)
        fr0 = np.zeros((FR, FC), np.float32)
        fr0[1:97, 2:194] = aup[0:96]
        fr0[97, 2:194] = aup[96]
        afr[0] = fr0.reshape(FF)
        fr1 = np.zeros((FR, FC), np.float32)
        fr1[1:97, 2:194] = aup[96:192]
        fr1[0, 2:194] = aup[95]
        afr[1] = fr1.reshape(FF)
        af = np.zeros((18, FF), np.float32)
        for h in range(2):
            for t in range(9):
                dlt = _delta(t)
                src = afr[h]
                dst = np.zeros(FF, np.float32)
                if dlt >= 0:
                    dst[:FF - dlt] = src[dlt:]
                else:
                    dst[-dlt:] = src[:FF + dlt]
                af[h * 9 + t] = dst
        maps.append(dict(
            xb=_pad_frame(x[b], BF16),
            x8=_pad_frame(x[b], F8),
            wkd=np.ascontiguousarray(wkd).astype(F8),
            wp8=np.ascontiguousarray(wp8).astype(F8),
            wpb=np.ascontiguousarray(wpb),
            wa=np.ascontiguousarray(wa),
            af=np.ascontiguousarray(af).astype(F8),
            bias=bias))
    return maps


def _diag128(v):
    out = np.zeros((128, 128), np.float32)
    out[np.arange(128), np.arange(128)] = v
    return out


def kernel(**inputs):
    from concourse.bass_utils import run_bass_kernel_spmd

    x = np.asarray(inputs['x'], np.float32)
    d = np.asarray(inputs['d'], np.float32)
    in_maps = _host_precompute(x, d, inputs)

    if 'nc' not in _CACHE:
        _CACHE['nc'] = _build_nc()
    nc = _CACHE['nc']

    try:
        res = run_bass_kernel_spmd(nc, in_maps, list(range(B)))
    except Exception:
        # transient NRT_EXEC_UNIT_UNRECOVERABLE observed on back-to-back
        # runs; a single retry is free and often clears it
        res = run_bass_kernel_spmd(nc, in_maps, list(range(B)))
    out = np.stack([np.asarray(res.results[i]['y'], np.float32).reshape(C, H, W)
                    for i in range(B)])
    return out


# revision 80
# speedup vs baseline: 1.0107x; 1.0107x over previous
"""Trainium2 Bass kernel for the dynamic-attention-block CNN (nn_DAB).

Data-parallel over batch: 8 samples -> 8 NeuronCores. Each core runs the
full per-sample network with activations resident in SBUF as padded
"frames": 128 partitions = 64 channels x 2 image halves, each half a
98x196 zero-padded row-major frame (rows -1..96 / 95..192 of the 192x192
image, cols -2..193).

Conv structure (all single 128-partition matmuls; the two image halves
ride in one instruction via block-diagonal weights):
  - 3x3 convs and dynamic depthwise convs run in fp8e4m3 DoubleRow mode:
    taps are processed in pairs (lhsT [128,2,128], rhs [128,2,N] with the
    pair dim striding between the two tap offsets), 5 passes per conv.
  - the 1x1 convs are also fp8 DoubleRow via stride-0 pairs (second pair
    slot zero-weighted), halving their cost; their rhs t1 = prelu(dw
    psum) is stored in fp8 with the dw kernel pre-scale KS=16 riding
    along (prelu is positively homogeneous; KS=16 keeps t1 in fp8 range)
    and divided out of the 1x1 weights.
  - channel-attention gates are bf16 diagonal matmuls accumulating
    x*att into the same PSUM group as the 1x1 (no vector gating pass);
    the conv3 residual +x is instead an in-place GPSIMD tensor_tensor
    add on the drained f32 output tiles.
  - the additive 32x32-upsampled map is folded into conv2 as one fp8
    DoubleRow pass: 18 partitions hold the 9 tap-shifted copies of the
    upsampled map per half, weights are the channel-summed conv2 taps.

Schedule: two software-pipelined wavefront blocks with per-stage tile
rotation (stage k starts at tile 2k, so each stage's wrap tiles land
long after the previous stage produced their halo rows, which are
DMA'd as soon as source tiles 47/0 drain):
  block A = da1 + conv1 (lag 8) + conv2 (lag 16)
  block B = da2 + conv3 (lag 8)
Interleaving adjacent stages keeps every drain engine below the PE
rate (a lone da stage is drain-bound) and lets conv3's f32 output
stream (~26us of serialized DMA-engine time) overlap da2's compute.
PSUM drains alternate between Act (1-op activation with scale+bias)
and DVE (two-scalar tensor_scalar + SBUF-only prelu via
scalar_tensor_tensor, which cannot read PSUM); the fp8 frame copy for
da2 alternates DVE/GPSIMD.  Output rides in 2-tile (4-row) groups (the
final 8 tiles in two dedicated 4-tile groups, shortening the
end-of-stream chain), two DMAs per group on the otherwise-idle sync
queue.  Input DMAs ride one
queue in consumption order (small kd1/1x1 weight pack first, then
interleaved x8/xb row chunks) so da1 never outruns the stream.
"""

import sys

for _p in ("/opt/trn_rl_repo", "/root/.axon_site/_ro/pypackages"):
    if _p not in sys.path:
        sys.path.insert(0, _p)

import numpy as np
import ml_dtypes

BF16 = ml_dtypes.bfloat16
F8 = ml_dtypes.float8_e4m3

B, C, H, W = 8, 64, 192, 192
HW = H * W
FR, FC = 98, 196          # frame rows / cols per half
FF = FR * FC              # frame elems per partition
ALPHA = 0.1               # leaky slope
WS = 16.0                 # fp8 conv weight pre-scale
KS = 16.0                 # fp8 dw kernel pre-scale (also bounds fp8 t1)
SC = 256.0                # da-stage PSUM scale: psB = SC*(1x1 out + att*x)

# bias pack columns (the *W columns are pre-scaled by WS / SC for DVE
# drains, which add the bias before the descale multiply; Act drains
# scale first)
(BI_B1, BI_B1W, BI_B2, BI_B2W, BI_B3, BI_B3W,
 BI_CB1, BI_CB1S, BI_CB2, BI_CB2S) = range(10)

# DoubleRow tap pairing: (tap_a, tap_b) with taps t = 3*dy + dx,
# delta(t) = (dy-1)*FC + (dx-1).  5 passes cover all 9 taps; the last
# pass's second slot has zero weight (stride 0 keeps the read in-bounds).
PAIRS = [(0, 1), (3, 4), (6, 7), (2, 5), (8, None)]

# packed fp8 weight slots: wkd carries what da1's first tiles need
# (kd1 + the two 1x1 stride-0 DoubleRow pairs) as a small early DMA;
# wp8 carries the rest
WK_KD1, WK_CW1, WK_CW2 = 0, 5, 6          # rows in wkd [128, 7, 2, 128]
WP_W1, WP_W2, WP_KD2, WP_W3 = range(4)    # 5-row slots in wp8
# packed bf16 weight slots in wpackb
WB_G1, WB_G2, WB_RID = range(3)

NTILE = 48
ROT = 2                   # per-stage tile-order rotation

_CACHE = {}


def _delta(t):
    return (t // 3 - 1) * FC + (t % 3 - 1)


def _build_nc():
    import concourse.bacc as bacc
    import concourse.mybir as mybir
    from concourse import tile

    f32 = mybir.dt.float32
    bf16 = mybir.dt.bfloat16
    f8 = mybir.dt.float8e4
    AF = mybir.ActivationFunctionType
    ALU = mybir.AluOpType
    DR = mybir.MatmulPerfMode.DoubleRow

    nc = bacc.Bacc("TRN2", target_bir_lowering=False, debug=False, num_devices=8)

    xb_d = nc.dram_tensor("xb", [128, FF], bf16, kind="ExternalInput").ap()
    x8_d = nc.dram_tensor("x8", [128, FF], f8, kind="ExternalInput").ap()
    wkd_d = nc.dram_tensor("wkd", [128, 7, 2, 128], f8, kind="ExternalInput").ap()
    wp8_d = nc.dram_tensor("wp8", [128, 20, 2, 128], f8, kind="ExternalInput").ap()
    wpb_d = nc.dram_tensor("wpb", [128, 3, 128], bf16, kind="ExternalInput").ap()
    wa_d = nc.dram_tensor("wa", [18, 2, 128], f8, kind="ExternalInput").ap()
    af_d = nc.dram_tensor("af", [18, FF], f8, kind="ExternalInput").ap()
    bias_d = nc.dram_tensor("bias", [128, 10], f32, kind="ExternalInput").ap()
    y_d = nc.dram_tensor("y", [C, HW], f32, kind="ExternalOutput").ap()

    # row-pair sweep tiles: 48 tiles of 2 image rows (392 frame cols)
    qt = [((2 * i + 1) * FC + 2, 2 * FC) for i in range(NTILE)]

    from contextlib import ExitStack
    with tile.TileContext(nc) as tc, ExitStack() as ctx:
        wpool = ctx.enter_context(tc.tile_pool(name="w", bufs=1))
        fbpool = ctx.enter_context(tc.tile_pool(name="fb", bufs=2))
        f8pool = ctx.enter_context(tc.tile_pool(name="f8", bufs=4))
        t1p = ctx.enter_context(tc.tile_pool(name="t1", bufs=4))
        tmpp = ctx.enter_context(tc.tile_pool(name="tmp", bufs=4))
        outp = ctx.enter_context(tc.tile_pool(name="outp", bufs=4))
        outb = ctx.enter_context(tc.tile_pool(name="outb", bufs=2))
        psA = ctx.enter_context(tc.tile_pool(name="psA", bufs=4, space="PSUM"))
        psB = ctx.enter_context(tc.tile_pool(name="psB", bufs=4, space="PSUM"))

        # ---- weights / constants to SBUF (batched DMAs) ----
        wkd = wpool.tile([128, 7, 2, 128], f8, tag="wkd")
        wp8 = wpool.tile([128, 20, 2, 128], f8, tag="wp8")
        wpb = wpool.tile([128, 3, 128], bf16, tag="wpb")
        wa = wpool.tile([18, 2, 128], f8, tag="wa")
        af = wpool.tile([18, FF], f8, tag="af")
        bias = wpool.tile([128, 10], f32, tag="bias")

        def w8(slot):
            return wp8[:, 5 * slot:5 * slot + 5, :, :]

        def wk5(row):
            return wkd[:, row:row + 5, :, :]

        def wcw(row):
            return wkd[:, row, :, :]

        def wb(slot):
            return wpb[:, slot, :]

        def cold(col):
            return bias[:, col:col + 1]

        # ---- input frames (host pre-padded); all bulk input DMAs ride
        # one queue in consumption order: weights first, then x8/xb
        # chunks interleaved by row coverage so da1 never outruns the
        # stream (transfers serialize on the DMA engines, so order is
        # what matters; the af map is only needed from conv2 onward) ----
        Xb = fbpool.tile([128, FF], bf16, tag="fb")
        X8 = f8pool.tile([128, FF], f8, tag="f8")
        O1 = f8pool.tile([128, FF], f8, tag="f8")
        O2 = f8pool.tile([128, FF], f8, tag="f8")
        O3b = fbpool.tile([128, FF], bf16, tag="fb")
        O38 = f8pool.tile([128, FF], f8, tag="f8")
        nc.sync.dma_start(out=wkd[...], in_=wkd_d)
        nc.gpsimd.dma_start(out=wpb[...], in_=wpb_d)
        nc.gpsimd.dma_start(out=wa[...], in_=wa_d)
        nc.gpsimd.dma_start(out=bias[...], in_=bias_d)
        # first fp8 rows split out so da1's tile 0 can start sooner
        nc.sync.dma_start(out=X8[:, 0:1176], in_=x8_d[:, 0:1176])
        nchunk = 8
        step = (FF + nchunk - 1) // nchunk
        for k in range(nchunk):
            c0, c1 = k * step, min((k + 1) * step, FF)
            if k == 0:
                nc.sync.dma_start(out=X8[:, 1176:c1], in_=x8_d[:, 1176:c1])
            else:
                nc.sync.dma_start(out=X8[:, c0:c1], in_=x8_d[:, c0:c1])
            nc.sync.dma_start(out=Xb[:, c0:c1], in_=xb_d[:, c0:c1])
            if k == 1:
                nc.sync.dma_start(out=wp8[...], in_=wp8_d)
        nc.scalar.dma_start(out=af[...], in_=af_d)

        # ---- PE warmup: throwaway matmuls while the input DMAs stream
        # in; keeps the p-state ramp finished before real work ----
        wrm = wpool.tile([128, 128], bf16, tag="wrm")
        nc.vector.memset(wrm[:, :], 0.0)
        pw = psA.tile([128, 128], f32, tag="psA")
        for _ in range(40):
            nc.tensor.matmul(pw[:, :], wrm[:, :], wrm[:, :],
                             start=True, stop=True, skip_group_check=True)

        def v3(m):
            return m[:, :].rearrange("p (a b) -> p a b", b=FC)

        # one-time pad zeroing for frame buffers not filled by host DMA.
        # Interior writes never touch pads again, so pads stay zero across
        # all later reuses of these pool buffers.
        for m in (O1, O2, O3b, O38):
            mv = v3(m)
            nc.gpsimd.memset(mv[0:64, 0, :], 0.0)
            nc.gpsimd.memset(mv[64:128, FR - 1, :], 0.0)
            nc.gpsimd.memset(mv[:, :, 0:2], 0.0)
            nc.gpsimd.memset(mv[:, :, FC - 2:FC], 0.0)

        def halo_a(m):
            # half1 top halo row (img 95) <- half0 frame row 96, src tile 47
            mv = v3(m)
            nc.sync.dma_start(out=mv[64:128, 0, :], in_=mv[0:64, 96, :])

        def halo_b(m):
            # half0 bottom halo row (img 96) <- half1 frame row 1, src tile 0
            mv = v3(m)
            nc.sync.dma_start(out=mv[0:64, FR - 1, :], in_=mv[64:128, 1, :])

        def maybe_halo(t, frames):
            if t == 47:
                for m in frames:
                    halo_a(m)
            elif t == 0:
                for m in frames:
                    halo_b(m)

        def order(stage):
            s = (ROT * stage) % NTILE
            return [(s + i) % NTILE for i in range(NTILE)]

        def dr_rhs(m8, q, n, pair):
            ta, tb = pair
            base = q + _delta(ta)
            stride = 0 if tb is None else _delta(tb) - _delta(ta)
            n = min(n, FF - base - max(stride, 0))
            r = m8[:, base:base + 1].copy()
            r.ap[1] = [stride, 2]
            r.ap.append([1, n])
            return r, n

        def conv_dr(ps, wsb, m8, q, n, stop=True):
            # P0 (top-left taps) never clamps, so it is the start pass and
            # always covers the full tile; clamped later passes only lose
            # tail columns that are pad positions, never emitted.
            for p in range(5):
                rhs, np_ = dr_rhs(m8, q, n, PAIRS[p])
                nc.tensor.matmul(ps[:, :np_], wsb[:, p, :, :], rhs,
                                 start=(p == 0), stop=(stop and p == 4),
                                 perf_mode=DR, skip_group_check=True)

        def iview(dst, q):
            # interior-only view: rows of the pair, cols 2:194
            r = q // FC
            return v3(dst)[:, r:r + 2, 2:194]

        def pview(src_ps, n):
            return src_ps[:, :n].rearrange("p (a b) -> p a b", b=FC)[:, :, 0:192]

        # Per-stage drain-engine alternation: each stage's per-engine drain
        # rate must stay below the PE rate or the drain queue backlog
        # throttles PSUM-buffer reuse (and the next stage's drains behind
        # it in the same queue).  Act drains are 1 op (~511ns); DVE prelu
        # drains are 2 ops (~800ns) since scalar_tensor_tensor can't read
        # PSUM, but plain identity+bias drains are 1 DVE op (~533ns).

        def prelu_drain_split(dst, ps, q, n, bw_col, scale):
            # ((psum + S*b) * 1/S) on DVE (bf16 staging), then an
            # SBUF-only prelu into the frame on GPSIMD, which is idle
            # during the da stages (scalar_tensor_tensor cannot read
            # PSUM, and a 2-op drain on DVE alone outruns the PE rate)
            tm = tmpp.tile([128, 2 * FC], bf16, tag="tm")
            tv = tm[:, :n].rearrange("p (a b) -> p a b", b=FC)[:, :, 0:192]
            nc.vector.tensor_scalar(tv, pview(ps, n), cold(bw_col),
                                    1.0 / scale, op0=ALU.add, op1=ALU.mult)
            nc.vector.scalar_tensor_tensor(iview(dst, q), tv, ALPHA, tv,
                                           op0=ALU.mult, op1=ALU.max)

        def make_da_stage(stage, inb, in8, kd_ap, cw_row, g_slot, cb_col,
                          cbs_col, out8):
            # Returns a step(i) driver, i in [0, NTILE+2): software-
            # pipelined by two tiles so the in-order PE queue rides out
            # the psA->Act t1->cw dependency chain (~1us).
            kd, cw, g = kd_ap, wcw(cw_row), wb(g_slot)
            tiles = order(stage)
            pipe = []

            def tail(prev, i):
                t1, q, n, t = prev
                pb = psB.tile([128, 2 * FC], f32, tag="psB")
                r = t1[:, 0:1].copy()
                r.ap[1] = [0, 2]
                r.ap.append([1, n])
                nc.tensor.matmul(pb[:, :n], cw, r, start=True, stop=False,
                                 perf_mode=DR, skip_group_check=True)
                nc.tensor.matmul(pb[:, :n], g, inb[:, q:q + n],
                                 start=False, stop=True, skip_group_check=True)
                if i % 5 in (0, 2):
                    nc.scalar.activation(iview(out8, q), pview(pb, n),
                                         AF.Prelu, scale=1.0 / SC,
                                         bias=cold(cb_col), alpha=ALPHA)
                else:
                    prelu_drain_split(out8, pb, q, n, cbs_col, SC)
                maybe_halo(t, (out8,))

            def step(i):
                if i < NTILE:
                    t = tiles[i]
                    q, n = qt[t]
                    pa = psA.tile([128, 2 * FC], f32, tag="psA")
                    conv_dr(pa, kd, in8, q, n)
                    if len(pipe) == 2:
                        tail(pipe.pop(0), i)
                    # t1 = prelu(psA) in fp8; the KS dw-weight scale rides
                    # along (prelu is positively homogeneous, KS=16 keeps
                    # the scaled values inside fp8 range) and is divided
                    # out of the fp8 1x1 weights on the host.
                    t1 = t1p.tile([128, 2 * FC], f8, tag="t1")
                    nc.scalar.activation(t1[:, :n], pa[:, :n], AF.Prelu,
                                         alpha=ALPHA)
                    pipe.append((t1, q, n, t))
                elif pipe:
                    tail(pipe.pop(0), i)

            return step

        # ---- network ----
        # da1 INTERLEAVED with conv1 (lag 8) and conv2 (lag 16): merging
        # the three stages' drain loads keeps Act and DVE below the
        # combined PE rate (da1 alone is drain-bound), and removes two
        # stage seams.  conv2 writes O38 while da1 still reads X8, so
        # O38 needs its own frame buffer (f8pool bufs=4).
        da1 = make_da_stage(0, Xb, X8, wk5(WK_KD1), WK_CW1, WB_G1, BI_CB1,
                            BI_CB1S, O1)
        LAG1, LAG2 = 8, 16
        c1tiles = order(1)
        c2tiles = order(2)
        for i in range(NTILE + LAG2):
            if i < NTILE + 2:
                da1(i)
            j = i - LAG1
            if 0 <= j < NTILE:
                t = c1tiles[j]
                q, n = qt[t]
                pa = psA.tile([128, 2 * FC], f32, tag="psA")
                conv_dr(pa, w8(WP_W1), O1, q, n)
                if j % 5 not in (1, 3):
                    nc.scalar.activation(iview(O2, q), pview(pa, n),
                                         AF.Prelu, scale=1.0 / WS,
                                         bias=cold(BI_B1), alpha=ALPHA)
                else:
                    prelu_drain_split(O2, pa, q, n, BI_B1W, WS)
                maybe_halo(t, (O2,))
            j2 = i - LAG2
            if j2 < 0:
                continue
            t = c2tiles[j2]
            q, n = qt[t]
            pa = psA.tile([128, 2 * FC], f32, tag="psA")
            conv_dr(pa, w8(WP_W2), O2, q, n, stop=False)
            r = af[:, q:q + 1].copy()
            r.ap[1] = [0, 2]
            r.ap.append([1, n])
            nc.tensor.matmul(pa[:, :n], wa[:, :, :], r,
                             start=False, stop=True, perf_mode=DR,
                             skip_group_check=True)
            if j2 % 2 == 0:
                nc.scalar.activation(iview(O3b, q), pview(pa, n), AF.Identity,
                                     scale=1.0 / WS, bias=cold(BI_B2))
                nc.gpsimd.tensor_copy(O38[:, q:q + n], O3b[:, q:q + n])
            else:
                nc.vector.tensor_scalar(iview(O3b, q), pview(pa, n),
                                        cold(BI_B2W), 1.0 / WS,
                                        op0=ALU.add, op1=ALU.mult)
                nc.vector.tensor_copy(O38[:, q:q + n], O3b[:, q:q + n])
            maybe_halo(t, (O38,))

        # ---- da2 INTERLEAVED with conv3 + residual (lag 8 positions):
        # conv3's outputs start streaming to DRAM ~30us earlier, so the
        # ~26us serialized output-DMA stream overlaps da2's compute
        # instead of trailing it, and the two stages' Act/DVE drain loads
        # merge to ~70-75% utilization each.  conv3: x (bf16, scaled by
        # WS via diag weights) and WS*b3 accumulate straight into PSUM;
        # drain with 1/WS.  Output rides in 4-tile (8-row) groups so each
        # group is just two DMAs, both on the otherwise-idle sync queue.
        # order(4) starts at tile 8 (4-aligned), so groups of 4
        # successive positions cover 4 consecutive tiles across the wrap.
        O4 = f8pool.tile([128, FF], f8, tag="f8")
        da2 = make_da_stage(3, O3b, O38, w8(WP_KD2), WK_CW2, WB_G2, BI_CB2,
                            BI_CB2S, O4)
        LAG3 = 8
        c3tiles = order(4)
        ot = None
        for i in range(NTILE + LAG3):
            if i < NTILE + 2:
                da2(i)
            j = i - LAG3
            if j < 0:
                continue
            t = c3tiles[j]
            q, n = qt[t]
            pa = psA.tile([128, 2 * FC], f32, tag="psA")
            conv_dr(pa, w8(WP_W3), O4, q, n)
            gsz = 2 if j < 40 else 4  # bigger final groups: fewer DMAs
            k = j % gsz                   # in the end-of-stream chain
            if k == 0:
                if gsz == 2:
                    ot = outp.tile([128, 4, 192], f32, tag="ot")
                else:
                    ot = outb.tile([128, 8, 192], f32, tag="otb")
                g0 = q // FC - 1  # image row of the group's first pair
            otv = ot[:, 2 * k:2 * k + 2, :]
            if j % 2 == 0 or j >= 40:
                nc.scalar.activation(otv, pview(pa, n), AF.Identity,
                                     scale=1.0 / WS, bias=cold(BI_B3))
            else:
                nc.vector.tensor_scalar(otv, pview(pa, n),
                                        cold(BI_B3W), 1.0 / WS,
                                        op0=ALU.add, op1=ALU.mult)
            # residual add in-place (mixed f32 += bf16): frees the PE from
            # the diag-matmul residual pass.  GPSIMD carries it during the
            # da2 overlap; in the final solo stretch the drain->resid
            # latency chain gates the last output DMAs, so keep it on the
            # faster DVE there (Act takes all the drains).
            if j >= 40:
                nc.vector.tensor_tensor(otv, otv, iview(Xb, q), op=ALU.add)
            else:
                nc.gpsimd.tensor_tensor(otv, otv, iview(Xb, q), op=ALU.add)
            if k == gsz - 1:
                gr = 2 * gsz
                nc.sync.dma_start(
                    out=y_d[:, g0 * 192:(g0 + gr) * 192]
                    .rearrange("p (r c) -> p r c", c=192),
                    in_=ot[0:64, :, :])
                nc.sync.dma_start(
                    out=y_d[:, (96 + g0) * 192:(96 + g0 + gr) * 192]
                    .rearrange("p (r c) -> p r c", c=192),
                    in_=ot[64:128, :, :])

    nc.compile()
    return nc


def _pad_frame(xb, dtype):
    """(64,192,192) fp32 -> (128, FR*FC) dual-half padded frame."""
    fr = np.zeros((128, FR, FC), np.float32)
    fr[0:64, 1:97, 2:194] = xb[:, 0:96, :]
    fr[0:64, 97, 2:194] = xb[:, 96, :]
    fr[64:128, 1:97, 2:194] = xb[:, 96:192, :]
    fr[64:128, 0, 2:194] = xb[:, 95, :]
    return np.ascontiguousarray(fr.reshape(128, FF)).astype(dtype)


def _leaky_np(v):
    return np.where(v >= 0, v, ALPHA * v)


def _host_precompute(x, d, p):
    """Build per-core input maps. p: dict of raw weight arrays."""
    d = d.astype(np.float64)
    kern = {}
    att = {}
    for i in (1, 2):
        kw1, kw2 = p[f'da{i}_kw1'].astype(np.float64), p[f'da{i}_kw2'].astype(np.float64)
        ca1, ca2 = p[f'da{i}_ca1'].astype(np.float64), p[f'da{i}_ca2'].astype(np.float64)
        kern[i] = _leaky_np(d @ kw1.T) @ kw2.T          # (B, 576) [c*9+t]
        z = _leaky_np(d @ ca1.T) @ ca2.T
        att[i] = 1.0 / (1.0 + np.exp(-z))               # (B, 64)
    a32 = _leaky_np(d @ p['add_w1'].astype(np.float64).T) @ \
        p['add_w2'].astype(np.float64).T                # (B, 1024)

    cidx = np.arange(128) % 64

    def convw_dr(w):
        # (O, C, 3, 3) fp32 -> [128, 5, 2, 128] f8 block-diag DoubleRow taps
        wq = (w.astype(np.float32) * WS).astype(F8).astype(np.float32)
        wt = wq.transpose(1, 2, 3, 0).reshape(64, 9, 64)  # [c, t, o]
        out = np.zeros((128, 5, 2, 128), np.float32)
        for pi, (ta, tb) in enumerate(PAIRS):
            blk = np.zeros((64, 2, 64), np.float32)
            blk[:, 0, :] = wt[:, ta, :]
            if tb is not None:
                blk[:, 1, :] = wt[:, tb, :]
            out[0:64, pi, :, 0:64] = blk
            out[64:128, pi, :, 64:128] = blk
        return out.astype(F8)

    def cw_dr(w):
        # (O, C) -> [128, 2, 128] fp8 stride-0 DoubleRow pair: slot 0 is
        # the block-diag 1x1 weight scaled by SC/KS (t1 carries KS, the
        # drain divides SC back out), slot 1 is zero.
        out = np.zeros((128, 2, 128), np.float32)
        out[0:64, 0, 0:64] = w.T * (SC / KS)
        out[64:128, 0, 64:128] = w.T * (SC / KS)
        return out.astype(F8)

    # fp8 packed conv/dw weights (per-sample kd slots filled below)
    w1 = convw_dr(p['conv1_w'])
    w2 = convw_dr(p['conv2_w'])
    w3 = convw_dr(p['conv3_w'])
    cw1 = cw_dr(p['da1_cw'])
    cw2 = cw_dr(p['da2_cw'])

    # additive-map conv weights: wa[(h,t), 0, o_col] = WS * sum_c w2[o,c,t]
    w2sum = p['conv2_w'].astype(np.float64).sum(axis=1).reshape(64, 9)  # [o, t]
    wa = np.zeros((18, 2, 128), np.float32)
    for h in range(2):
        for t in range(9):
            wa[h * 9 + t, 0, h * 64:(h + 1) * 64] = WS * w2sum[:, t]
    wa = wa.astype(F8)

    rid = _diag128(np.full(128, WS, np.float32))

    maps = []
    for b in range(B):
        kd = {}
        for i in (1, 2):
            kc = (kern[i][b].reshape(64, 9).astype(np.float32) * KS) \
                .astype(F8).astype(np.float32)           # [c, t]
            kdl = np.zeros((128, 5, 2, 128), np.float32)
            for pi, (ta, tb) in enumerate(PAIRS):
                kdl[np.arange(128), pi, 0, np.arange(128)] = kc[cidx, ta]
                if tb is not None:
                    kdl[np.arange(128), pi, 1, np.arange(128)] = kc[cidx, tb]
            kd[i] = kdl.astype(F8)
        g = {i: _diag128(SC * att[i][b][cidx]) for i in (1, 2)}
        wkd = np.concatenate(
            [kd[1], cw1.reshape(128, 1, 2, 128),
             cw2.reshape(128, 1, 2, 128)], axis=1)
        wp8 = np.concatenate([w1, w2, kd[2], w3], axis=1)
        wpb = np.stack([g[1], g[2], rid], axis=1).astype(BF16)
        bias = np.zeros((128, 10), np.float32)
        bias[:, BI_B1] = p['conv1_b'][cidx]
        bias[:, BI_B1W] = WS * p['conv1_b'][cidx]
        bias[:, BI_B2] = p['conv2_b'][cidx]
        bias[:, BI_B2W] = WS * p['conv2_b'][cidx]
        bias[:, BI_B3] = p['conv3_b'][cidx]
        bias[:, BI_B3W] = WS * p['conv3_b'][cidx]
        bias[:, BI_CB1] = p['da1_cb'][cidx]
        bias[:, BI_CB1S] = SC * p['da1_cb'][cidx]
        bias[:, BI_CB2] = p['da2_cb'][cidx]
        bias[:, BI_CB2S] = SC * p['da2_cb'][cidx]

        # additive map frames: 18 partitions = 2 halves x 9 tap shifts
        a = a32[b].astype(np.float32).reshape(32, 32)
        aup = a[np.arange(192) // 6][:, np.arange(192) // 6]  # (192,192)
        afr = np.zeros((2, FF), np.float32)
        fr0 = np.zeros((FR, FC), np.float32)
        fr0[1:97, 2:194] = aup[0:96]
        fr0[97, 2:194] = aup[96]
        afr[0] = fr0.reshape(FF)
        fr1 = np.zeros((FR, FC), np.float32)
        fr1[1:97, 2:194] = aup[96:192]
        fr1[0, 2:194] = aup[95]
        afr[1] = fr1.reshape(FF)
        af = np.zeros((18, FF), np.float32)
        for h in range(2):
            for t in range(9):
                dlt = _delta(t)
                src = afr[h]
                dst = np.zeros(FF, np.float32)
                if dlt >= 0:
                    dst[:FF - dlt] = src[dlt:]
                else:
                    dst[-dlt:] = src[:FF + dlt]
                af[h * 9 + t] = dst
        maps.append(dict(
            xb=_pad_frame(x[b], BF16),
            x8=_pad_frame(x[b], F8),
            wkd=np.ascontiguousarray(wkd).astype(F8),
            wp8=np.ascontiguousarray(wp8).astype(F8),
            wpb=np.ascontiguousarray(wpb),
            wa=np.ascontiguousarray(wa),
            af=np.ascontiguousarray(af).astype(F8),
            bias=bias))
    return maps


def _diag128(v):
    out = np.zeros((128, 128), np.float32)
    out[np.arange(128), np.arange(128)] = v
    return out


def kernel(**inputs):
    from concourse.bass_utils import run_bass_kernel_spmd

    x = np.asarray(inputs['x'], np.float32)
    d = np.asarray(inputs['d'], np.float32)
    in_maps = _host_precompute(x, d, inputs)

    if 'nc' not in _CACHE:
        _CACHE['nc'] = _build_nc()
    nc = _CACHE['nc']

    try:
        res = run_bass_kernel_spmd(nc, in_maps, list(range(B)))
    except Exception:
        # transient NRT_EXEC_UNIT_UNRECOVERABLE observed on back-to-back
        # runs; a single retry is free and often clears it
        res = run_bass_kernel_spmd(nc, in_maps, list(range(B)))
    out = np.stack([np.asarray(res.results[i]['y'], np.float32).reshape(C, H, W)
                    for i in range(B)])
    return out


# revision 81
# speedup vs baseline: 1.0114x; 1.0007x over previous
"""Trainium2 Bass kernel for the dynamic-attention-block CNN (nn_DAB).

Data-parallel over batch: 8 samples -> 8 NeuronCores. Each core runs the
full per-sample network with activations resident in SBUF as padded
"frames": 128 partitions = 64 channels x 2 image halves, each half a
98x196 zero-padded row-major frame (rows -1..96 / 95..192 of the 192x192
image, cols -2..193).

Conv structure (all single 128-partition matmuls; the two image halves
ride in one instruction via block-diagonal weights):
  - 3x3 convs and dynamic depthwise convs run in fp8e4m3 DoubleRow mode:
    taps are processed in pairs (lhsT [128,2,128], rhs [128,2,N] with the
    pair dim striding between the two tap offsets), 5 passes per conv.
  - the 1x1 convs are also fp8 DoubleRow via stride-0 pairs (second pair
    slot zero-weighted), halving their cost; their rhs t1 = prelu(dw
    psum) is stored in fp8 with the dw kernel pre-scale KS=16 riding
    along (prelu is positively homogeneous; KS=16 keeps t1 in fp8 range)
    and divided out of the 1x1 weights.
  - channel-attention gates are bf16 diagonal matmuls accumulating
    x*att into the same PSUM group as the 1x1 (no vector gating pass);
    the conv3 residual +x is instead an in-place GPSIMD tensor_tensor
    add on the drained f32 output tiles.
  - the additive 32x32-upsampled map is folded into conv2 as one fp8
    DoubleRow pass: 18 partitions hold the 9 tap-shifted copies of the
    upsampled map per half, weights are the channel-summed conv2 taps.

Schedule: two software-pipelined wavefront blocks with per-stage tile
rotation (stage k starts at tile 2k, so each stage's wrap tiles land
long after the previous stage produced their halo rows, which are
DMA'd as soon as source tiles 47/0 drain):
  block A = da1 + conv1 (lag 8) + conv2 (lag 16)
  block B = da2 + conv3 (lag 8)
Interleaving adjacent stages keeps every drain engine below the PE
rate (a lone da stage is drain-bound) and lets conv3's f32 output
stream (~26us of serialized DMA-engine time) overlap da2's compute.
PSUM drains alternate between Act (1-op activation with scale+bias)
and DVE (two-scalar tensor_scalar + SBUF-only prelu via
scalar_tensor_tensor, which cannot read PSUM); the fp8 frame copy for
da2 alternates DVE/GPSIMD.  Output rides in 2-tile (4-row) groups (the
final 8 tiles in two dedicated 4-tile groups, shortening the
end-of-stream chain), two DMAs per group on the otherwise-idle sync
queue.  Input DMAs ride one
queue in consumption order (small kd1/1x1 weight pack first, then
interleaved x8/xb row chunks) so da1 never outruns the stream.
"""

import sys

for _p in ("/opt/trn_rl_repo", "/root/.axon_site/_ro/pypackages"):
    if _p not in sys.path:
        sys.path.insert(0, _p)

import numpy as np
import ml_dtypes

BF16 = ml_dtypes.bfloat16
F8 = ml_dtypes.float8_e4m3

B, C, H, W = 8, 64, 192, 192
HW = H * W
FR, FC = 98, 196          # frame rows / cols per half
FF = FR * FC              # frame elems per partition
ALPHA = 0.1               # leaky slope
WS = 16.0                 # fp8 conv weight pre-scale
KS = 16.0                 # fp8 dw kernel pre-scale (also bounds fp8 t1)
SC = 256.0                # da-stage PSUM scale: psB = SC*(1x1 out + att*x)

# bias pack columns (the *W columns are pre-scaled by WS / SC for DVE
# drains, which add the bias before the descale multiply; Act drains
# scale first)
(BI_B1, BI_B1W, BI_B2, BI_B2W, BI_B3, BI_B3W,
 BI_CB1, BI_CB1S, BI_CB2, BI_CB2S) = range(10)

# DoubleRow tap pairing: (tap_a, tap_b) with taps t = 3*dy + dx,
# delta(t) = (dy-1)*FC + (dx-1).  5 passes cover all 9 taps; the last
# pass's second slot has zero weight (stride 0 keeps the read in-bounds).
PAIRS = [(0, 1), (3, 4), (6, 7), (2, 5), (8, None)]

# packed fp8 weight slots: wkd carries what da1's first tiles need
# (kd1 + the two 1x1 stride-0 DoubleRow pairs) as a small early DMA;
# wp8 carries the rest
WK_KD1, WK_CW1, WK_CW2 = 0, 5, 6          # rows in wkd [128, 7, 2, 128]
WP_W1, WP_W2, WP_KD2, WP_W3 = range(4)    # 5-row slots in wp8
# packed bf16 weight slots in wpackb
WB_G1, WB_G2, WB_RID = range(3)

NTILE = 48
ROT = 2                   # per-stage tile-order rotation

_CACHE = {}


def _delta(t):
    return (t // 3 - 1) * FC + (t % 3 - 1)


def _build_nc():
    import concourse.bacc as bacc
    import concourse.mybir as mybir
    from concourse import tile

    f32 = mybir.dt.float32
    bf16 = mybir.dt.bfloat16
    f8 = mybir.dt.float8e4
    AF = mybir.ActivationFunctionType
    ALU = mybir.AluOpType
    DR = mybir.MatmulPerfMode.DoubleRow

    nc = bacc.Bacc("TRN2", target_bir_lowering=False, debug=False, num_devices=8)

    xb_d = nc.dram_tensor("xb", [128, FF], bf16, kind="ExternalInput").ap()
    x8_d = nc.dram_tensor("x8", [128, FF], f8, kind="ExternalInput").ap()
    wkd_d = nc.dram_tensor("wkd", [128, 7, 2, 128], f8, kind="ExternalInput").ap()
    wp8_d = nc.dram_tensor("wp8", [128, 20, 2, 128], f8, kind="ExternalInput").ap()
    wpb_d = nc.dram_tensor("wpb", [128, 3, 128], bf16, kind="ExternalInput").ap()
    wa_d = nc.dram_tensor("wa", [18, 2, 128], f8, kind="ExternalInput").ap()
    af_d = nc.dram_tensor("af", [18, FF], f8, kind="ExternalInput").ap()
    bias_d = nc.dram_tensor("bias", [128, 10], f32, kind="ExternalInput").ap()
    y_d = nc.dram_tensor("y", [C, HW], f32, kind="ExternalOutput").ap()

    # row-pair sweep tiles: 48 tiles of 2 image rows (392 frame cols)
    qt = [((2 * i + 1) * FC + 2, 2 * FC) for i in range(NTILE)]

    from contextlib import ExitStack
    with tile.TileContext(nc) as tc, ExitStack() as ctx:
        wpool = ctx.enter_context(tc.tile_pool(name="w", bufs=1))
        fbpool = ctx.enter_context(tc.tile_pool(name="fb", bufs=2))
        f8pool = ctx.enter_context(tc.tile_pool(name="f8", bufs=4))
        t1p = ctx.enter_context(tc.tile_pool(name="t1", bufs=4))
        tmpp = ctx.enter_context(tc.tile_pool(name="tmp", bufs=4))
        outp = ctx.enter_context(tc.tile_pool(name="outp", bufs=4))
        outb = ctx.enter_context(tc.tile_pool(name="outb", bufs=2))
        psA = ctx.enter_context(tc.tile_pool(name="psA", bufs=4, space="PSUM"))
        psB = ctx.enter_context(tc.tile_pool(name="psB", bufs=4, space="PSUM"))

        # ---- weights / constants to SBUF (batched DMAs) ----
        wkd = wpool.tile([128, 7, 2, 128], f8, tag="wkd")
        wp8 = wpool.tile([128, 20, 2, 128], f8, tag="wp8")
        wpb = wpool.tile([128, 3, 128], bf16, tag="wpb")
        wa = wpool.tile([18, 2, 128], f8, tag="wa")
        af = wpool.tile([18, FF], f8, tag="af")
        bias = wpool.tile([128, 10], f32, tag="bias")

        def w8(slot):
            return wp8[:, 5 * slot:5 * slot + 5, :, :]

        def wk5(row):
            return wkd[:, row:row + 5, :, :]

        def wcw(row):
            return wkd[:, row, :, :]

        def wb(slot):
            return wpb[:, slot, :]

        def cold(col):
            return bias[:, col:col + 1]

        # ---- input frames (host pre-padded); all bulk input DMAs ride
        # one queue in consumption order: weights first, then x8/xb
        # chunks interleaved by row coverage so da1 never outruns the
        # stream (transfers serialize on the DMA engines, so order is
        # what matters; the af map is only needed from conv2 onward) ----
        Xb = fbpool.tile([128, FF], bf16, tag="fb")
        X8 = f8pool.tile([128, FF], f8, tag="f8")
        O1 = f8pool.tile([128, FF], f8, tag="f8")
        O2 = f8pool.tile([128, FF], f8, tag="f8")
        O3b = fbpool.tile([128, FF], bf16, tag="fb")
        O38 = f8pool.tile([128, FF], f8, tag="f8")
        nc.sync.dma_start(out=wkd[...], in_=wkd_d)
        nc.gpsimd.dma_start(out=wpb[...], in_=wpb_d)
        nc.gpsimd.dma_start(out=wa[...], in_=wa_d)
        nc.gpsimd.dma_start(out=bias[...], in_=bias_d)
        # first fp8 rows split out so da1's tile 0 can start sooner
        nc.sync.dma_start(out=X8[:, 0:1176], in_=x8_d[:, 0:1176])
        nchunk = 8
        step = (FF + nchunk - 1) // nchunk
        for k in range(nchunk):
            c0, c1 = k * step, min((k + 1) * step, FF)
            if k == 0:
                nc.sync.dma_start(out=X8[:, 1176:c1], in_=x8_d[:, 1176:c1])
            else:
                nc.sync.dma_start(out=X8[:, c0:c1], in_=x8_d[:, c0:c1])
            nc.sync.dma_start(out=Xb[:, c0:c1], in_=xb_d[:, c0:c1])
            if k == 1:
                nc.sync.dma_start(out=wp8[...], in_=wp8_d)
        nc.scalar.dma_start(out=af[...], in_=af_d)

        # ---- PE warmup: throwaway matmuls while the input DMAs stream
        # in; keeps the p-state ramp finished before real work ----
        wrm = wpool.tile([128, 128], bf16, tag="wrm")
        nc.vector.memset(wrm[:, :], 0.0)
        pw = psA.tile([128, 128], f32, tag="psA")
        for _ in range(40):
            nc.tensor.matmul(pw[:, :], wrm[:, :], wrm[:, :],
                             start=True, stop=True, skip_group_check=True)

        def v3(m):
            return m[:, :].rearrange("p (a b) -> p a b", b=FC)

        # one-time pad zeroing for frame buffers not filled by host DMA.
        # Interior writes never touch pads again, so pads stay zero across
        # all later reuses of these pool buffers.
        for m in (O1, O2, O3b, O38):
            mv = v3(m)
            nc.gpsimd.memset(mv[0:64, 0, :], 0.0)
            nc.gpsimd.memset(mv[64:128, FR - 1, :], 0.0)
            nc.gpsimd.memset(mv[:, :, 0:2], 0.0)
            nc.gpsimd.memset(mv[:, :, FC - 2:FC], 0.0)

        def halo_a(m):
            # half1 top halo row (img 95) <- half0 frame row 96, src tile 47
            mv = v3(m)
            nc.sync.dma_start(out=mv[64:128, 0, :], in_=mv[0:64, 96, :])

        def halo_b(m):
            # half0 bottom halo row (img 96) <- half1 frame row 1, src tile 0
            mv = v3(m)
            nc.sync.dma_start(out=mv[0:64, FR - 1, :], in_=mv[64:128, 1, :])

        def maybe_halo(t, frames):
            if t == 47:
                for m in frames:
                    halo_a(m)
            elif t == 0:
                for m in frames:
                    halo_b(m)

        def order(stage):
            s = (ROT * stage) % NTILE
            return [(s + i) % NTILE for i in range(NTILE)]

        def dr_rhs(m8, q, n, pair):
            ta, tb = pair
            base = q + _delta(ta)
            stride = 0 if tb is None else _delta(tb) - _delta(ta)
            n = min(n, FF - base - max(stride, 0))
            r = m8[:, base:base + 1].copy()
            r.ap[1] = [stride, 2]
            r.ap.append([1, n])
            return r, n

        def conv_dr(ps, wsb, m8, q, n, stop=True):
            # P0 (top-left taps) never clamps, so it is the start pass and
            # always covers the full tile; clamped later passes only lose
            # tail columns that are pad positions, never emitted.
            for p in range(5):
                rhs, np_ = dr_rhs(m8, q, n, PAIRS[p])
                nc.tensor.matmul(ps[:, :np_], wsb[:, p, :, :], rhs,
                                 start=(p == 0), stop=(stop and p == 4),
                                 perf_mode=DR, skip_group_check=True)

        def iview(dst, q):
            # interior-only view: rows of the pair, cols 2:194
            r = q // FC
            return v3(dst)[:, r:r + 2, 2:194]

        def pview(src_ps, n):
            return src_ps[:, :n].rearrange("p (a b) -> p a b", b=FC)[:, :, 0:192]

        # Per-stage drain-engine alternation: each stage's per-engine drain
        # rate must stay below the PE rate or the drain queue backlog
        # throttles PSUM-buffer reuse (and the next stage's drains behind
        # it in the same queue).  Act drains are 1 op (~511ns); DVE prelu
        # drains are 2 ops (~800ns) since scalar_tensor_tensor can't read
        # PSUM, but plain identity+bias drains are 1 DVE op (~533ns).

        def prelu_drain_split(dst, ps, q, n, bw_col, scale):
            # ((psum + S*b) * 1/S) on DVE (bf16 staging), then an
            # SBUF-only prelu into the frame on GPSIMD, which is idle
            # during the da stages (scalar_tensor_tensor cannot read
            # PSUM, and a 2-op drain on DVE alone outruns the PE rate)
            tm = tmpp.tile([128, 2 * FC], bf16, tag="tm")
            tv = tm[:, :n].rearrange("p (a b) -> p a b", b=FC)[:, :, 0:192]
            nc.vector.tensor_scalar(tv, pview(ps, n), cold(bw_col),
                                    1.0 / scale, op0=ALU.add, op1=ALU.mult)
            nc.vector.scalar_tensor_tensor(iview(dst, q), tv, ALPHA, tv,
                                           op0=ALU.mult, op1=ALU.max)

        def make_da_stage(stage, inb, in8, kd_ap, cw_row, g_slot, cb_col,
                          cbs_col, out8, act_every=5):
            # Returns a step(i) driver, i in [0, NTILE+2): software-
            # pipelined by two tiles so the in-order PE queue rides out
            # the psA->Act t1->cw dependency chain (~1us).
            kd, cw, g = kd_ap, wcw(cw_row), wb(g_slot)
            tiles = order(stage)
            pipe = []

            def tail(prev, i):
                t1, q, n, t = prev
                pb = psB.tile([128, 2 * FC], f32, tag="psB")
                r = t1[:, 0:1].copy()
                r.ap[1] = [0, 2]
                r.ap.append([1, n])
                nc.tensor.matmul(pb[:, :n], cw, r, start=True, stop=False,
                                 perf_mode=DR, skip_group_check=True)
                nc.tensor.matmul(pb[:, :n], g, inb[:, q:q + n],
                                 start=False, stop=True, skip_group_check=True)
                if (i % 5 in (0, 2)) if act_every == 5 else (i % 2 == 0):
                    nc.scalar.activation(iview(out8, q), pview(pb, n),
                                         AF.Prelu, scale=1.0 / SC,
                                         bias=cold(cb_col), alpha=ALPHA)
                else:
                    prelu_drain_split(out8, pb, q, n, cbs_col, SC)
                maybe_halo(t, (out8,))

            def step(i):
                if i < NTILE:
                    t = tiles[i]
                    q, n = qt[t]
                    pa = psA.tile([128, 2 * FC], f32, tag="psA")
                    conv_dr(pa, kd, in8, q, n)
                    if len(pipe) == 2:
                        tail(pipe.pop(0), i)
                    # t1 = prelu(psA) in fp8; the KS dw-weight scale rides
                    # along (prelu is positively homogeneous, KS=16 keeps
                    # the scaled values inside fp8 range) and is divided
                    # out of the fp8 1x1 weights on the host.
                    t1 = t1p.tile([128, 2 * FC], f8, tag="t1")
                    nc.scalar.activation(t1[:, :n], pa[:, :n], AF.Prelu,
                                         alpha=ALPHA)
                    pipe.append((t1, q, n, t))
                elif pipe:
                    tail(pipe.pop(0), i)

            return step

        # ---- network ----
        # da1 INTERLEAVED with conv1 (lag 8) and conv2 (lag 16): merging
        # the three stages' drain loads keeps Act and DVE below the
        # combined PE rate (da1 alone is drain-bound), and removes two
        # stage seams.  conv2 writes O38 while da1 still reads X8, so
        # O38 needs its own frame buffer (f8pool bufs=4).
        da1 = make_da_stage(0, Xb, X8, wk5(WK_KD1), WK_CW1, WB_G1, BI_CB1,
                            BI_CB1S, O1, act_every=2)
        LAG1, LAG2 = 8, 16
        c1tiles = order(1)
        c2tiles = order(2)
        for i in range(NTILE + LAG2):
            if i < NTILE + 2:
                da1(i)
            j = i - LAG1
            if 0 <= j < NTILE:
                t = c1tiles[j]
                q, n = qt[t]
                pa = psA.tile([128, 2 * FC], f32, tag="psA")
                conv_dr(pa, w8(WP_W1), O1, q, n)
                if j % 5 not in (1, 3):
                    nc.scalar.activation(iview(O2, q), pview(pa, n),
                                         AF.Prelu, scale=1.0 / WS,
                                         bias=cold(BI_B1), alpha=ALPHA)
                else:
                    prelu_drain_split(O2, pa, q, n, BI_B1W, WS)
                maybe_halo(t, (O2,))
            j2 = i - LAG2
            if j2 < 0:
                continue
            t = c2tiles[j2]
            q, n = qt[t]
            pa = psA.tile([128, 2 * FC], f32, tag="psA")
            conv_dr(pa, w8(WP_W2), O2, q, n, stop=False)
            r = af[:, q:q + 1].copy()
            r.ap[1] = [0, 2]
            r.ap.append([1, n])
            nc.tensor.matmul(pa[:, :n], wa[:, :, :], r,
                             start=False, stop=True, perf_mode=DR,
                             skip_group_check=True)
            if j2 % 2 == 0:
                nc.scalar.activation(iview(O3b, q), pview(pa, n), AF.Identity,
                                     scale=1.0 / WS, bias=cold(BI_B2))
                nc.gpsimd.tensor_copy(O38[:, q:q + n], O3b[:, q:q + n])
            else:
                nc.vector.tensor_scalar(iview(O3b, q), pview(pa, n),
                                        cold(BI_B2W), 1.0 / WS,
                                        op0=ALU.add, op1=ALU.mult)
                nc.vector.tensor_copy(O38[:, q:q + n], O3b[:, q:q + n])
            maybe_halo(t, (O38,))

        # ---- da2 INTERLEAVED with conv3 + residual (lag 8 positions):
        # conv3's outputs start streaming to DRAM ~30us earlier, so the
        # ~26us serialized output-DMA stream overlaps da2's compute
        # instead of trailing it, and the two stages' Act/DVE drain loads
        # merge to ~70-75% utilization each.  conv3: x (bf16, scaled by
        # WS via diag weights) and WS*b3 accumulate straight into PSUM;
        # drain with 1/WS.  Output rides in 4-tile (8-row) groups so each
        # group is just two DMAs, both on the otherwise-idle sync queue.
        # order(4) starts at tile 8 (4-aligned), so groups of 4
        # successive positions cover 4 consecutive tiles across the wrap.
        O4 = f8pool.tile([128, FF], f8, tag="f8")
        da2 = make_da_stage(3, O3b, O38, w8(WP_KD2), WK_CW2, WB_G2, BI_CB2,
                            BI_CB2S, O4)
        LAG3 = 8
        c3tiles = order(4)
        ot = None
        for i in range(NTILE + LAG3):
            if i < NTILE + 2:
                da2(i)
            j = i - LAG3
            if j < 0:
                continue
            t = c3tiles[j]
            q, n = qt[t]
            pa = psA.tile([128, 2 * FC], f32, tag="psA")
            conv_dr(pa, w8(WP_W3), O4, q, n)
            gsz = 2 if j < 40 else 4  # bigger final groups: fewer DMAs
            k = j % gsz                   # in the end-of-stream chain
            if k == 0:
                if gsz == 2:
                    ot = outp.tile([128, 4, 192], f32, tag="ot")
                else:
                    ot = outb.tile([128, 8, 192], f32, tag="otb")
                g0 = q // FC - 1  # image row of the group's first pair
            otv = ot[:, 2 * k:2 * k + 2, :]
            if j % 2 == 0 or j >= 40:
                nc.scalar.activation(otv, pview(pa, n), AF.Identity,
                                     scale=1.0 / WS, bias=cold(BI_B3))
            else:
                nc.vector.tensor_scalar(otv, pview(pa, n),
                                        cold(BI_B3W), 1.0 / WS,
                                        op0=ALU.add, op1=ALU.mult)
            # residual add in-place (mixed f32 += bf16): frees the PE from
            # the diag-matmul residual pass.  GPSIMD carries it during the
            # da2 overlap; in the final solo stretch the drain->resid
            # latency chain gates the last output DMAs, so keep it on the
            # faster DVE there (Act takes all the drains).
            if j >= 40:
                nc.vector.tensor_tensor(otv, otv, iview(Xb, q), op=ALU.add)
            else:
                nc.gpsimd.tensor_tensor(otv, otv, iview(Xb, q), op=ALU.add)
            if k == gsz - 1:
                gr = 2 * gsz
                nc.sync.dma_start(
                    out=y_d[:, g0 * 192:(g0 + gr) * 192]
                    .rearrange("p (r c) -> p r c", c=192),
                    in_=ot[0:64, :, :])
                nc.sync.dma_start(
                    out=y_d[:, (96 + g0) * 192:(96 + g0 + gr) * 192]
                    .rearrange("p (r c) -> p r c", c=192),
                    in_=ot[64:128, :, :])

    nc.compile()
    return nc


def _pad_frame(xb, dtype):
    """(64,192,192) fp32 -> (128, FR*FC) dual-half padded frame."""
    fr = np.zeros((128, FR, FC), np.float32)
    fr[0:64, 1:97, 2:194] = xb[:, 0:96, :]
    fr[0:64, 97, 2:194] = xb[:, 96, :]
    fr[64:128, 1:97, 2:194] = xb[:, 96:192, :]
    fr[64:128, 0, 2:194] = xb[:, 95, :]
    return np.ascontiguousarray(fr.reshape(128, FF)).astype(dtype)


def _leaky_np(v):
    return np.where(v >= 0, v, ALPHA * v)


def _host_precompute(x, d, p):
    """Build per-core input maps. p: dict of raw weight arrays."""
    d = d.astype(np.float64)
    kern = {}
    att = {}
    for i in (1, 2):
        kw1, kw2 = p[f'da{i}_kw1'].astype(np.float64), p[f'da{i}_kw2'].astype(np.float64)
        ca1, ca2 = p[f'da{i}_ca1'].astype(np.float64), p[f'da{i}_ca2'].astype(np.float64)
        kern[i] = _leaky_np(d @ kw1.T) @ kw2.T          # (B, 576) [c*9+t]
        z = _leaky_np(d @ ca1.T) @ ca2.T
        att[i] = 1.0 / (1.0 + np.exp(-z))               # (B, 64)
    a32 = _leaky_np(d @ p['add_w1'].astype(np.float64).T) @ \
        p['add_w2'].astype(np.float64).T                # (B, 1024)

    cidx = np.arange(128) % 64

    def convw_dr(w):
        # (O, C, 3, 3) fp32 -> [128, 5, 2, 128] f8 block-diag DoubleRow taps
        wq = (w.astype(np.float32) * WS).astype(F8).astype(np.float32)
        wt = wq.transpose(1, 2, 3, 0).reshape(64, 9, 64)  # [c, t, o]
        out = np.zeros((128, 5, 2, 128), np.float32)
        for pi, (ta, tb) in enumerate(PAIRS):
            blk = np.zeros((64, 2, 64), np.float32)
            blk[:, 0, :] = wt[:, ta, :]
            if tb is not None:
                blk[:, 1, :] = wt[:, tb, :]
            out[0:64, pi, :, 0:64] = blk
            out[64:128, pi, :, 64:128] = blk
        return out.astype(F8)

    def cw_dr(w):
        # (O, C) -> [128, 2, 128] fp8 stride-0 DoubleRow pair: slot 0 is
        # the block-diag 1x1 weight scaled by SC/KS (t1 carries KS, the
        # drain divides SC back out), slot 1 is zero.
        out = np.zeros((128, 2, 128), np.float32)
        out[0:64, 0, 0:64] = w.T * (SC / KS)
        out[64:128, 0, 64:128] = w.T * (SC / KS)
        return out.astype(F8)

    # fp8 packed conv/dw weights (per-sample kd slots filled below)
    w1 = convw_dr(p['conv1_w'])
    w2 = convw_dr(p['conv2_w'])
    w3 = convw_dr(p['conv3_w'])
    cw1 = cw_dr(p['da1_cw'])
    cw2 = cw_dr(p['da2_cw'])

    # additive-map conv weights: wa[(h,t), 0, o_col] = WS * sum_c w2[o,c,t]
    w2sum = p['conv2_w'].astype(np.float64).sum(axis=1).reshape(64, 9)  # [o, t]
    wa = np.zeros((18, 2, 128), np.float32)
    for h in range(2):
        for t in range(9):
            wa[h * 9 + t, 0, h * 64:(h + 1) * 64] = WS * w2sum[:, t]
    wa = wa.astype(F8)

    rid = _diag128(np.full(128, WS, np.float32))

    maps = []
    for b in range(B):
        kd = {}
        for i in (1, 2):
            kc = (kern[i][b].reshape(64, 9).astype(np.float32) * KS) \
                .astype(F8).astype(np.float32)           # [c, t]
            kdl = np.zeros((128, 5, 2, 128), np.float32)
            for pi, (ta, tb) in enumerate(PAIRS):
                kdl[np.arange(128), pi, 0, np.arange(128)] = kc[cidx, ta]
                if tb is not None:
                    kdl[np.arange(128), pi, 1, np.arange(128)] = kc[cidx, tb]
            kd[i] = kdl.astype(F8)
        g = {i: _diag128(SC * att[i][b][cidx]) for i in (1, 2)}
        wkd = np.concatenate(
            [kd[1], cw1.reshape(128, 1, 2, 128),
             cw2.reshape(128, 1, 2, 128)], axis=1)
        wp8 = np.concatenate([w1, w2, kd[2], w3], axis=1)
        wpb = np.stack([g[1], g[2], rid], axis=1).astype(BF16)
        bias = np.zeros((128, 10), np.float32)
        bias[:, BI_B1] = p['conv1_b'][cidx]
        bias[:, BI_B1W] = WS * p['conv1_b'][cidx]
        bias[:, BI_B2] = p['conv2_b'][cidx]
        bias[:, BI_B2W] = WS * p['conv2_b'][cidx]
        bias[:, BI_B3] = p['conv3_b'][cidx]
        bias[:, BI_B3W] = WS * p['conv3_b'][cidx]
        bias[:, BI_CB1] = p['da1_cb'][cidx]
        bias[:, BI_CB1S] = SC * p['da1_cb'][cidx]
        bias[:, BI_CB2] = p['da2_cb'][cidx]
        bias[:, BI_CB2S] = SC * p['da2_cb'][cidx]

        # additive map frames: 18 partitions = 2 halves x 9 tap shifts
        a = a32[b].astype(np.float32).reshape(32, 32)
        aup = a[np.arange(192) // 6][:, np.arange(192) // 6]  # (192,192)
        afr = np.zeros((2, FF), np.float32)
        fr0 = np.zeros((FR, FC), np.float32)
        fr0[1:97, 2:194] = aup[0:96]
        fr0[97, 2:194] = aup[96]
        afr[0] = fr0.reshape(FF)
        fr1 = np.zeros((FR, FC), np.float32)
        fr1[1:97, 2:194] = aup[96:192]
        fr1[0, 2:194] = aup[95]
        afr[1] = fr1.reshape(FF)
        af = np.zeros((18, FF), np.float32)
        for h in range(2):
            for t in range(9):
                dlt = _delta(t)
                src = afr[h]
                dst = np.zeros(FF, np.float32)
                if dlt >= 0:
                    dst[:FF - dlt] = src[dlt:]
                else:
                    dst[-dlt:] = src[:FF + dlt]
                af[h * 9 + t] = dst
        maps.append(dict(
            xb=_pad_frame(x[b], BF16),
            x8=_pad_frame(x[b], F8),
            wkd=np.ascontiguousarray(wkd).astype(F8),
            wp8=np.ascontiguousarray(wp8).astype(F8),
            wpb=np.ascontiguousarray(wpb),
            wa=np.ascontiguousarray(wa),
            af=np.ascontiguousarray(af).astype(F8),
            bias=bias))
    return maps


def _diag128(v):
    out = np.zeros((128, 128), np.float32)
    out[np.arange(128), np.arange(128)] = v
    return out


def kernel(**inputs):
    from concourse.bass_utils import run_bass_kernel_spmd

    x = np.asarray(inputs['x'], np.float32)
    d = np.asarray(inputs['d'], np.float32)
    in_maps = _host_precompute(x, d, inputs)

    if 'nc' not in _CACHE:
        _CACHE['nc'] = _build_nc()
    nc = _CACHE['nc']

    try:
        res = run_bass_kernel_spmd(nc, in_maps, list(range(B)))
    except Exception:
        # transient NRT_EXEC_UNIT_UNRECOVERABLE observed on back-to-back
        # runs; a single retry is free and often clears it
        res = run_bass_kernel_spmd(nc, in_maps, list(range(B)))
    out = np.stack([np.asarray(res.results[i]['y'], np.float32).reshape(C, H, W)
                    for i in range(B)])
    return out


# revision 82
# speedup vs baseline: 1.0123x; 1.0009x over previous
"""Trainium2 Bass kernel for the dynamic-attention-block CNN (nn_DAB).

Data-parallel over batch: 8 samples -> 8 NeuronCores. Each core runs the
full per-sample network with activations resident in SBUF as padded
"frames": 128 partitions = 64 channels x 2 image halves, each half a
98x196 zero-padded row-major frame (rows -1..96 / 95..192 of the 192x192
image, cols -2..193).

Conv structure (all single 128-partition matmuls; the two image halves
ride in one instruction via block-diagonal weights):
  - 3x3 convs and dynamic depthwise convs run in fp8e4m3 DoubleRow mode:
    taps are processed in pairs (lhsT [128,2,128], rhs [128,2,N] with the
    pair dim striding between the two tap offsets), 5 passes per conv.
  - the 1x1 convs are also fp8 DoubleRow via stride-0 pairs (second pair
    slot zero-weighted), halving their cost; their rhs t1 = prelu(dw
    psum) is stored in fp8 with the dw kernel pre-scale KS=16 riding
    along (prelu is positively homogeneous; KS=16 keeps t1 in fp8 range)
    and divided out of the 1x1 weights.
  - channel-attention gates are bf16 diagonal matmuls accumulating
    x*att into the same PSUM group as the 1x1 (no vector gating pass);
    the conv3 residual +x is instead an in-place GPSIMD tensor_tensor
    add on the drained f32 output tiles.
  - the additive 32x32-upsampled map is folded into conv2 as one fp8
    DoubleRow pass: 18 partitions hold the 9 tap-shifted copies of the
    upsampled map per half, weights are the channel-summed conv2 taps.

Schedule: two software-pipelined wavefront blocks with per-stage tile
rotation (stage k starts at tile 2k, so each stage's wrap tiles land
long after the previous stage produced their halo rows, which are
DMA'd as soon as source tiles 47/0 drain):
  block A = da1 + conv1 (lag 8) + conv2 (lag 16)
  block B = da2 + conv3 (lag 8)
Interleaving adjacent stages keeps every drain engine below the PE
rate (a lone da stage is drain-bound) and lets conv3's f32 output
stream (~26us of serialized DMA-engine time) overlap da2's compute.
PSUM drains alternate between Act (1-op activation with scale+bias)
and DVE (two-scalar tensor_scalar + SBUF-only prelu via
scalar_tensor_tensor, which cannot read PSUM); the fp8 frame copy for
da2 alternates DVE/GPSIMD.  Output rides in 2-tile (4-row) groups (the
final 8 tiles in two dedicated 4-tile groups, shortening the
end-of-stream chain), two DMAs per group on the otherwise-idle sync
queue.  Input DMAs ride one
queue in consumption order (small kd1/1x1 weight pack first, then
interleaved x8/xb row chunks) so da1 never outruns the stream.
"""

import sys

for _p in ("/opt/trn_rl_repo", "/root/.axon_site/_ro/pypackages"):
    if _p not in sys.path:
        sys.path.insert(0, _p)

import numpy as np
import ml_dtypes

BF16 = ml_dtypes.bfloat16
F8 = ml_dtypes.float8_e4m3

B, C, H, W = 8, 64, 192, 192
HW = H * W
FR, FC = 98, 196          # frame rows / cols per half
FF = FR * FC              # frame elems per partition
ALPHA = 0.1               # leaky slope
WS = 16.0                 # fp8 conv weight pre-scale
KS = 16.0                 # fp8 dw kernel pre-scale (also bounds fp8 t1)
SC = 256.0                # da-stage PSUM scale: psB = SC*(1x1 out + att*x)

# bias pack columns (the *W columns are pre-scaled by WS / SC for DVE
# drains, which add the bias before the descale multiply; Act drains
# scale first)
(BI_B1, BI_B1W, BI_B2, BI_B2W, BI_B3, BI_B3W,
 BI_CB1, BI_CB1S, BI_CB2, BI_CB2S) = range(10)

# DoubleRow tap pairing: (tap_a, tap_b) with taps t = 3*dy + dx,
# delta(t) = (dy-1)*FC + (dx-1).  5 passes cover all 9 taps; the last
# pass's second slot has zero weight (stride 0 keeps the read in-bounds).
PAIRS = [(0, 1), (3, 4), (6, 7), (2, 5), (8, None)]

# packed fp8 weight slots: wkd carries what da1's first tiles need
# (kd1 + the two 1x1 stride-0 DoubleRow pairs) as a small early DMA;
# wp8 carries the rest
WK_KD1, WK_CW1, WK_CW2 = 0, 5, 6          # rows in wkd [128, 7, 2, 128]
WP_W1, WP_W2, WP_KD2, WP_W3 = range(4)    # 5-row slots in wp8
# packed bf16 weight slots in wpackb
WB_G1, WB_G2, WB_RID = range(3)

NTILE = 48
ROT = 2                   # per-stage tile-order rotation

_CACHE = {}


def _delta(t):
    return (t // 3 - 1) * FC + (t % 3 - 1)


def _build_nc():
    import concourse.bacc as bacc
    import concourse.mybir as mybir
    from concourse import tile

    f32 = mybir.dt.float32
    bf16 = mybir.dt.bfloat16
    f8 = mybir.dt.float8e4
    AF = mybir.ActivationFunctionType
    ALU = mybir.AluOpType
    DR = mybir.MatmulPerfMode.DoubleRow

    nc = bacc.Bacc("TRN2", target_bir_lowering=False, debug=False, num_devices=8)

    xb_d = nc.dram_tensor("xb", [128, FF], bf16, kind="ExternalInput").ap()
    x8_d = nc.dram_tensor("x8", [128, FF], f8, kind="ExternalInput").ap()
    wkd_d = nc.dram_tensor("wkd", [128, 7, 2, 128], f8, kind="ExternalInput").ap()
    wp8_d = nc.dram_tensor("wp8", [128, 20, 2, 128], f8, kind="ExternalInput").ap()
    wpb_d = nc.dram_tensor("wpb", [128, 3, 128], bf16, kind="ExternalInput").ap()
    wa_d = nc.dram_tensor("wa", [18, 2, 128], f8, kind="ExternalInput").ap()
    af_d = nc.dram_tensor("af", [18, FF], f8, kind="ExternalInput").ap()
    bias_d = nc.dram_tensor("bias", [128, 10], f32, kind="ExternalInput").ap()
    y_d = nc.dram_tensor("y", [C, HW], f32, kind="ExternalOutput").ap()

    # row-pair sweep tiles: 48 tiles of 2 image rows (392 frame cols)
    qt = [((2 * i + 1) * FC + 2, 2 * FC) for i in range(NTILE)]

    from contextlib import ExitStack
    with tile.TileContext(nc) as tc, ExitStack() as ctx:
        wpool = ctx.enter_context(tc.tile_pool(name="w", bufs=1))
        fbpool = ctx.enter_context(tc.tile_pool(name="fb", bufs=2))
        f8pool = ctx.enter_context(tc.tile_pool(name="f8", bufs=4))
        t1p = ctx.enter_context(tc.tile_pool(name="t1", bufs=4))
        tmpp = ctx.enter_context(tc.tile_pool(name="tmp", bufs=4))
        outp = ctx.enter_context(tc.tile_pool(name="outp", bufs=4))
        outb = ctx.enter_context(tc.tile_pool(name="outb", bufs=2))
        psA = ctx.enter_context(tc.tile_pool(name="psA", bufs=4, space="PSUM"))
        psB = ctx.enter_context(tc.tile_pool(name="psB", bufs=4, space="PSUM"))

        # ---- weights / constants to SBUF (batched DMAs) ----
        wkd = wpool.tile([128, 7, 2, 128], f8, tag="wkd")
        wp8 = wpool.tile([128, 20, 2, 128], f8, tag="wp8")
        wpb = wpool.tile([128, 3, 128], bf16, tag="wpb")
        wa = wpool.tile([18, 2, 128], f8, tag="wa")
        af = wpool.tile([18, FF], f8, tag="af")
        bias = wpool.tile([128, 10], f32, tag="bias")

        def w8(slot):
            return wp8[:, 5 * slot:5 * slot + 5, :, :]

        def wk5(row):
            return wkd[:, row:row + 5, :, :]

        def wcw(row):
            return wkd[:, row, :, :]

        def wb(slot):
            return wpb[:, slot, :]

        def cold(col):
            return bias[:, col:col + 1]

        # ---- input frames (host pre-padded); all bulk input DMAs ride
        # one queue in consumption order: weights first, then x8/xb
        # chunks interleaved by row coverage so da1 never outruns the
        # stream (transfers serialize on the DMA engines, so order is
        # what matters; the af map is only needed from conv2 onward) ----
        Xb = fbpool.tile([128, FF], bf16, tag="fb")
        X8 = f8pool.tile([128, FF], f8, tag="f8")
        O1 = f8pool.tile([128, FF], f8, tag="f8")
        O2 = f8pool.tile([128, FF], f8, tag="f8")
        O3b = fbpool.tile([128, FF], bf16, tag="fb")
        O38 = f8pool.tile([128, FF], f8, tag="f8")
        nc.sync.dma_start(out=wkd[...], in_=wkd_d)
        nc.gpsimd.dma_start(out=wpb[...], in_=wpb_d)
        nc.gpsimd.dma_start(out=wa[...], in_=wa_d)
        nc.gpsimd.dma_start(out=bias[...], in_=bias_d)
        # first fp8 rows split out so da1's tile 0 can start sooner
        nc.sync.dma_start(out=X8[:, 0:1176], in_=x8_d[:, 0:1176])
        nchunk = 8
        step = (FF + nchunk - 1) // nchunk
        for k in range(nchunk):
            c0, c1 = k * step, min((k + 1) * step, FF)
            if k == 0:
                nc.sync.dma_start(out=X8[:, 1176:c1], in_=x8_d[:, 1176:c1])
            else:
                nc.sync.dma_start(out=X8[:, c0:c1], in_=x8_d[:, c0:c1])
            nc.sync.dma_start(out=Xb[:, c0:c1], in_=xb_d[:, c0:c1])
            if k == 1:
                nc.sync.dma_start(out=wp8[...], in_=wp8_d)
        nc.scalar.dma_start(out=af[...], in_=af_d)

        # ---- PE warmup: throwaway matmuls while the input DMAs stream
        # in; keeps the p-state ramp finished before real work ----
        wrm = wpool.tile([128, 128], bf16, tag="wrm")
        nc.vector.memset(wrm[:, :], 0.0)
        pw = psA.tile([128, 128], f32, tag="psA")
        for _ in range(40):
            nc.tensor.matmul(pw[:, :], wrm[:, :], wrm[:, :],
                             start=True, stop=True, skip_group_check=True)

        def v3(m):
            return m[:, :].rearrange("p (a b) -> p a b", b=FC)

        # one-time pad zeroing for frame buffers not filled by host DMA.
        # Interior writes never touch pads again, so pads stay zero across
        # all later reuses of these pool buffers.
        for m in (O1, O2, O3b, O38):
            mv = v3(m)
            nc.gpsimd.memset(mv[0:64, 0, :], 0.0)
            nc.gpsimd.memset(mv[64:128, FR - 1, :], 0.0)
            nc.gpsimd.memset(mv[:, :, 0:2], 0.0)
            nc.gpsimd.memset(mv[:, :, FC - 2:FC], 0.0)

        def halo_a(m):
            # half1 top halo row (img 95) <- half0 frame row 96, src tile 47
            mv = v3(m)
            nc.sync.dma_start(out=mv[64:128, 0, :], in_=mv[0:64, 96, :])

        def halo_b(m):
            # half0 bottom halo row (img 96) <- half1 frame row 1, src tile 0
            mv = v3(m)
            nc.sync.dma_start(out=mv[0:64, FR - 1, :], in_=mv[64:128, 1, :])

        def maybe_halo(t, frames):
            if t == 47:
                for m in frames:
                    halo_a(m)
            elif t == 0:
                for m in frames:
                    halo_b(m)

        def order(stage):
            s = (ROT * stage) % NTILE
            return [(s + i) % NTILE for i in range(NTILE)]

        def dr_rhs(m8, q, n, pair):
            ta, tb = pair
            base = q + _delta(ta)
            stride = 0 if tb is None else _delta(tb) - _delta(ta)
            n = min(n, FF - base - max(stride, 0))
            r = m8[:, base:base + 1].copy()
            r.ap[1] = [stride, 2]
            r.ap.append([1, n])
            return r, n

        def conv_dr(ps, wsb, m8, q, n, stop=True):
            # P0 (top-left taps) never clamps, so it is the start pass and
            # always covers the full tile; clamped later passes only lose
            # tail columns that are pad positions, never emitted.
            for p in range(5):
                rhs, np_ = dr_rhs(m8, q, n, PAIRS[p])
                nc.tensor.matmul(ps[:, :np_], wsb[:, p, :, :], rhs,
                                 start=(p == 0), stop=(stop and p == 4),
                                 perf_mode=DR, skip_group_check=True)

        def iview(dst, q):
            # interior-only view: rows of the pair, cols 2:194
            r = q // FC
            return v3(dst)[:, r:r + 2, 2:194]

        def pview(src_ps, n):
            return src_ps[:, :n].rearrange("p (a b) -> p a b", b=FC)[:, :, 0:192]

        # Per-stage drain-engine alternation: each stage's per-engine drain
        # rate must stay below the PE rate or the drain queue backlog
        # throttles PSUM-buffer reuse (and the next stage's drains behind
        # it in the same queue).  Act drains are 1 op (~511ns); DVE prelu
        # drains are 2 ops (~800ns) since scalar_tensor_tensor can't read
        # PSUM, but plain identity+bias drains are 1 DVE op (~533ns).

        def prelu_drain_split(dst, ps, q, n, bw_col, scale):
            # ((psum + S*b) * 1/S) on DVE (bf16 staging), then an
            # SBUF-only prelu into the frame on GPSIMD, which is idle
            # during the da stages (scalar_tensor_tensor cannot read
            # PSUM, and a 2-op drain on DVE alone outruns the PE rate)
            tm = tmpp.tile([128, 2 * FC], bf16, tag="tm")
            tv = tm[:, :n].rearrange("p (a b) -> p a b", b=FC)[:, :, 0:192]
            nc.vector.tensor_scalar(tv, pview(ps, n), cold(bw_col),
                                    1.0 / scale, op0=ALU.add, op1=ALU.mult)
            nc.vector.scalar_tensor_tensor(iview(dst, q), tv, ALPHA, tv,
                                           op0=ALU.mult, op1=ALU.max)

        def make_da_stage(stage, inb, in8, kd_ap, cw_row, g_slot, cb_col,
                          cbs_col, out8, act_every=5):
            # Returns a step(i) driver, i in [0, NTILE+2): software-
            # pipelined by two tiles so the in-order PE queue rides out
            # the psA->Act t1->cw dependency chain (~1us).
            kd, cw, g = kd_ap, wcw(cw_row), wb(g_slot)
            tiles = order(stage)
            pipe = []

            def tail(prev, i):
                t1, q, n, t = prev
                pb = psB.tile([128, 2 * FC], f32, tag="psB")
                r = t1[:, 0:1].copy()
                r.ap[1] = [0, 2]
                r.ap.append([1, n])
                nc.tensor.matmul(pb[:, :n], cw, r, start=True, stop=False,
                                 perf_mode=DR, skip_group_check=True)
                nc.tensor.matmul(pb[:, :n], g, inb[:, q:q + n],
                                 start=False, stop=True, skip_group_check=True)
                if (i % act_every == 0) if act_every != 5 else (i % 5 in (0, 2)):
                    nc.scalar.activation(iview(out8, q), pview(pb, n),
                                         AF.Prelu, scale=1.0 / SC,
                                         bias=cold(cb_col), alpha=ALPHA)
                else:
                    prelu_drain_split(out8, pb, q, n, cbs_col, SC)
                maybe_halo(t, (out8,))

            def step(i):
                if i < NTILE:
                    t = tiles[i]
                    q, n = qt[t]
                    pa = psA.tile([128, 2 * FC], f32, tag="psA")
                    conv_dr(pa, kd, in8, q, n)
                    if len(pipe) == 2:
                        tail(pipe.pop(0), i)
                    # t1 = prelu(psA) in fp8; the KS dw-weight scale rides
                    # along (prelu is positively homogeneous, KS=16 keeps
                    # the scaled values inside fp8 range) and is divided
                    # out of the fp8 1x1 weights on the host.
                    t1 = t1p.tile([128, 2 * FC], f8, tag="t1")
                    nc.scalar.activation(t1[:, :n], pa[:, :n], AF.Prelu,
                                         alpha=ALPHA)
                    pipe.append((t1, q, n, t))
                elif pipe:
                    tail(pipe.pop(0), i)

            return step

        # ---- network ----
        # da1 INTERLEAVED with conv1 (lag 8) and conv2 (lag 16): merging
        # the three stages' drain loads keeps Act and DVE below the
        # combined PE rate (da1 alone is drain-bound), and removes two
        # stage seams.  conv2 writes O38 while da1 still reads X8, so
        # O38 needs its own frame buffer (f8pool bufs=4).
        da1 = make_da_stage(0, Xb, X8, wk5(WK_KD1), WK_CW1, WB_G1, BI_CB1,
                            BI_CB1S, O1, act_every=2)
        LAG1, LAG2 = 8, 16
        c1tiles = order(1)
        c2tiles = order(2)
        for i in range(NTILE + LAG2):
            if i < NTILE + 2:
                da1(i)
            j = i - LAG1
            if 0 <= j < NTILE:
                t = c1tiles[j]
                q, n = qt[t]
                pa = psA.tile([128, 2 * FC], f32, tag="psA")
                conv_dr(pa, w8(WP_W1), O1, q, n)
                if j % 5 not in (1, 3):
                    nc.scalar.activation(iview(O2, q), pview(pa, n),
                                         AF.Prelu, scale=1.0 / WS,
                                         bias=cold(BI_B1), alpha=ALPHA)
                else:
                    prelu_drain_split(O2, pa, q, n, BI_B1W, WS)
                maybe_halo(t, (O2,))
            j2 = i - LAG2
            if j2 < 0:
                continue
            t = c2tiles[j2]
            q, n = qt[t]
            pa = psA.tile([128, 2 * FC], f32, tag="psA")
            conv_dr(pa, w8(WP_W2), O2, q, n, stop=False)
            r = af[:, q:q + 1].copy()
            r.ap[1] = [0, 2]
            r.ap.append([1, n])
            nc.tensor.matmul(pa[:, :n], wa[:, :, :], r,
                             start=False, stop=True, perf_mode=DR,
                             skip_group_check=True)
            if j2 % 2 == 0:
                nc.scalar.activation(iview(O3b, q), pview(pa, n), AF.Identity,
                                     scale=1.0 / WS, bias=cold(BI_B2))
                nc.gpsimd.tensor_copy(O38[:, q:q + n], O3b[:, q:q + n])
            else:
                nc.vector.tensor_scalar(iview(O3b, q), pview(pa, n),
                                        cold(BI_B2W), 1.0 / WS,
                                        op0=ALU.add, op1=ALU.mult)
                nc.vector.tensor_copy(O38[:, q:q + n], O3b[:, q:q + n])
            maybe_halo(t, (O38,))

        # ---- da2 INTERLEAVED with conv3 + residual (lag 8 positions):
        # conv3's outputs start streaming to DRAM ~30us earlier, so the
        # ~26us serialized output-DMA stream overlaps da2's compute
        # instead of trailing it, and the two stages' Act/DVE drain loads
        # merge to ~70-75% utilization each.  conv3: x (bf16, scaled by
        # WS via diag weights) and WS*b3 accumulate straight into PSUM;
        # drain with 1/WS.  Output rides in 4-tile (8-row) groups so each
        # group is just two DMAs, both on the otherwise-idle sync queue.
        # order(4) starts at tile 8 (4-aligned), so groups of 4
        # successive positions cover 4 consecutive tiles across the wrap.
        O4 = f8pool.tile([128, FF], f8, tag="f8")
        da2 = make_da_stage(3, O3b, O38, w8(WP_KD2), WK_CW2, WB_G2, BI_CB2,
                            BI_CB2S, O4, act_every=3)
        LAG3 = 8
        c3tiles = order(4)
        ot = None
        for i in range(NTILE + LAG3):
            if i < NTILE + 2:
                da2(i)
            j = i - LAG3
            if j < 0:
                continue
            t = c3tiles[j]
            q, n = qt[t]
            pa = psA.tile([128, 2 * FC], f32, tag="psA")
            conv_dr(pa, w8(WP_W3), O4, q, n)
            gsz = 2 if j < 40 else 4  # bigger final groups: fewer DMAs
            k = j % gsz                   # in the end-of-stream chain
            if k == 0:
                if gsz == 2:
                    ot = outp.tile([128, 4, 192], f32, tag="ot")
                else:
                    ot = outb.tile([128, 8, 192], f32, tag="otb")
                g0 = q // FC - 1  # image row of the group's first pair
            otv = ot[:, 2 * k:2 * k + 2, :]
            if j % 2 == 0 or j >= 40:
                nc.scalar.activation(otv, pview(pa, n), AF.Identity,
                                     scale=1.0 / WS, bias=cold(BI_B3))
            else:
                nc.vector.tensor_scalar(otv, pview(pa, n),
                                        cold(BI_B3W), 1.0 / WS,
                                        op0=ALU.add, op1=ALU.mult)
            # residual add in-place (mixed f32 += bf16): frees the PE from
            # the diag-matmul residual pass.  GPSIMD carries it during the
            # da2 overlap; in the final solo stretch the drain->resid
            # latency chain gates the last output DMAs, so keep it on the
            # faster DVE there (Act takes all the drains).
            if j >= 40:
                nc.vector.tensor_tensor(otv, otv, iview(Xb, q), op=ALU.add)
            else:
                nc.gpsimd.tensor_tensor(otv, otv, iview(Xb, q), op=ALU.add)
            if k == gsz - 1:
                gr = 2 * gsz
                nc.sync.dma_start(
                    out=y_d[:, g0 * 192:(g0 + gr) * 192]
                    .rearrange("p (r c) -> p r c", c=192),
                    in_=ot[0:64, :, :])
                nc.sync.dma_start(
                    out=y_d[:, (96 + g0) * 192:(96 + g0 + gr) * 192]
                    .rearrange("p (r c) -> p r c", c=192),
                    in_=ot[64:128, :, :])

    nc.compile()
    return nc


def _pad_frame(xb, dtype):
    """(64,192,192) fp32 -> (128, FR*FC) dual-half padded frame."""
    fr = np.zeros((128, FR, FC), np.float32)
    fr[0:64, 1:97, 2:194] = xb[:, 0:96, :]
    fr[0:64, 97, 2:194] = xb[:, 96, :]
    fr[64:128, 1:97, 2:194] = xb[:, 96:192, :]
    fr[64:128, 0, 2:194] = xb[:, 95, :]
    return np.ascontiguousarray(fr.reshape(128, FF)).astype(dtype)


def _leaky_np(v):
    return np.where(v >= 0, v, ALPHA * v)


def _host_precompute(x, d, p):
    """Build per-core input maps. p: dict of raw weight arrays."""
    d = d.astype(np.float64)
    kern = {}
    att = {}
    for i in (1, 2):
        kw1, kw2 = p[f'da{i}_kw1'].astype(np.float64), p[f'da{i}_kw2'].astype(np.float64)
        ca1, ca2 = p[f'da{i}_ca1'].astype(np.float64), p[f'da{i}_ca2'].astype(np.float64)
        kern[i] = _leaky_np(d @ kw1.T) @ kw2.T          # (B, 576) [c*9+t]
        z = _leaky_np(d @ ca1.T) @ ca2.T
        att[i] = 1.0 / (1.0 + np.exp(-z))               # (B, 64)
    a32 = _leaky_np(d @ p['add_w1'].astype(np.float64).T) @ \
        p['add_w2'].astype(np.float64).T                # (B, 1024)

    cidx = np.arange(128) % 64

    def convw_dr(w):
        # (O, C, 3, 3) fp32 -> [128, 5, 2, 128] f8 block-diag DoubleRow taps
        wq = (w.astype(np.float32) * WS).astype(F8).astype(np.float32)
        wt = wq.transpose(1, 2, 3, 0).reshape(64, 9, 64)  # [c, t, o]
        out = np.zeros((128, 5, 2, 128), np.float32)
        for pi, (ta, tb) in enumerate(PAIRS):
            blk = np.zeros((64, 2, 64), np.float32)
            blk[:, 0, :] = wt[:, ta, :]
            if tb is not None:
                blk[:, 1, :] = wt[:, tb, :]
            out[0:64, pi, :, 0:64] = blk
            out[64:128, pi, :, 64:128] = blk
        return out.astype(F8)

    def cw_dr(w):
        # (O, C) -> [128, 2, 128] fp8 stride-0 DoubleRow pair: slot 0 is
        # the block-diag 1x1 weight scaled by SC/KS (t1 carries KS, the
        # drain divides SC back out), slot 1 is zero.
        out = np.zeros((128, 2, 128), np.float32)
        out[0:64, 0, 0:64] = w.T * (SC / KS)
        out[64:128, 0, 64:128] = w.T * (SC / KS)
        return out.astype(F8)

    # fp8 packed conv/dw weights (per-sample kd slots filled below)
    w1 = convw_dr(p['conv1_w'])
    w2 = convw_dr(p['conv2_w'])
    w3 = convw_dr(p['conv3_w'])
    cw1 = cw_dr(p['da1_cw'])
    cw2 = cw_dr(p['da2_cw'])

    # additive-map conv weights: wa[(h,t), 0, o_col] = WS * sum_c w2[o,c,t]
    w2sum = p['conv2_w'].astype(np.float64).sum(axis=1).reshape(64, 9)  # [o, t]
    wa = np.zeros((18, 2, 128), np.float32)
    for h in range(2):
        for t in range(9):
            wa[h * 9 + t, 0, h * 64:(h + 1) * 64] = WS * w2sum[:, t]
    wa = wa.astype(F8)

    rid = _diag128(np.full(128, WS, np.float32))

    maps = []
    for b in range(B):
        kd = {}
        for i in (1, 2):
            kc = (kern[i][b].reshape(64, 9).astype(np.float32) * KS) \
                .astype(F8).astype(np.float32)           # [c, t]
            kdl = np.zeros((128, 5, 2, 128), np.float32)
            for pi, (ta, tb) in enumerate(PAIRS):
                kdl[np.arange(128), pi, 0, np.arange(128)] = kc[cidx, ta]
                if tb is not None:
                    kdl[np.arange(128), pi, 1, np.arange(128)] = kc[cidx, tb]
            kd[i] = kdl.astype(F8)
        g = {i: _diag128(SC * att[i][b][cidx]) for i in (1, 2)}
        wkd = np.concatenate(
            [kd[1], cw1.reshape(128, 1, 2, 128),
             cw2.reshape(128, 1, 2, 128)], axis=1)
        wp8 = np.concatenate([w1, w2, kd[2], w3], axis=1)
        wpb = np.stack([g[1], g[2], rid], axis=1).astype(BF16)
        bias = np.zeros((128, 10), np.float32)
        bias[:, BI_B1] = p['conv1_b'][cidx]
        bias[:, BI_B1W] = WS * p['conv1_b'][cidx]
        bias[:, BI_B2] = p['conv2_b'][cidx]
        bias[:, BI_B2W] = WS * p['conv2_b'][cidx]
        bias[:, BI_B3] = p['conv3_b'][cidx]
        bias[:, BI_B3W] = WS * p['conv3_b'][cidx]
        bias[:, BI_CB1] = p['da1_cb'][cidx]
        bias[:, BI_CB1S] = SC * p['da1_cb'][cidx]
        bias[:, BI_CB2] = p['da2_cb'][cidx]
        bias[:, BI_CB2S] = SC * p['da2_cb'][cidx]

        # additive map frames: 18 partitions = 2 halves x 9 tap shifts
        a = a32[b].astype(np.float32).reshape(32, 32)
        aup = a[np.arange(192) // 6][:, np.arange(192) // 6]  # (192,192)
        afr = np.zeros((2, FF), np.float32)
        fr0 = np.zeros((FR, FC), np.float32)
        fr0[1:97, 2:194] = aup[0:96]
        fr0[97, 2:194] = aup[96]
        afr[0] = fr0.reshape(FF)
        fr1 = np.zeros((FR, FC), np.float32)
        fr1[1:97, 2:194] = aup[96:192]
        fr1[0, 2:194] = aup[95]
        afr[1] = fr1.reshape(FF)
        af = np.zeros((18, FF), np.float32)
        for h in range(2):
            for t in range(9):
                dlt = _delta(t)
                src = afr[h]
                dst = np.zeros(FF, np.float32)
                if dlt >= 0:
                    dst[:FF - dlt] = src[dlt:]
                else:
                    dst[-dlt:] = src[:FF + dlt]
                af[h * 9 + t] = dst
        maps.append(dict(
            xb=_pad_frame(x[b], BF16),
            x8=_pad_frame(x[b], F8),
            wkd=np.ascontiguousarray(wkd).astype(F8),
            wp8=np.ascontiguousarray(wp8).astype(F8),
            wpb=np.ascontiguousarray(wpb),
            wa=np.ascontiguousarray(wa),
            af=np.ascontiguousarray(af).astype(F8),
            bias=bias))
    return maps


def _diag128(v):
    out = np.zeros((128, 128), np.float32)
    out[np.arange(128), np.arange(128)] = v
    return out


def kernel(**inputs):
    from concourse.bass_utils import run_bass_kernel_spmd

    x = np.asarray(inputs['x'], np.float32)
    d = np.asarray(inputs['d'], np.float32)
    in_maps = _host_precompute(x, d, inputs)

    if 'nc' not in _CACHE:
        _CACHE['nc'] = _build_nc()
    nc = _CACHE['nc']

    try:
        res = run_bass_kernel_spmd(nc, in_maps, list(range(B)))
    except Exception:
        # transient NRT_EXEC_UNIT_UNRECOVERABLE observed on back-to-back
        # runs; a single retry is free and often clears it
        res = run_bass_kernel_spmd(nc, in_maps, list(range(B)))
    out = np.stack([np.asarray(res.results[i]['y'], np.float32).reshape(C, H, W)
                    for i in range(B)])
    return out
